# revision 26
# baseline (speedup 1.0000x reference)
"""Trainium2 Bass kernel for nn_CATransformer1 (XCiT-style channel-attention block).

v2: bf16 matmuls, LN centering folded into host-prepared weights, S-gram
weighted by inv-variance on the q side, transpose-free G build, fused
ffn1/ffn2 pipeline with F=512 moving tiles.

Sharding: data-parallel over batch. 16 images / 8 cores = 2 images per core.

Math (per image, x [C=384, N=4096]):
  LN1 gamma and the mean-subtraction are folded into the QKV weights on the
  host: W' = W*g1 - rowmean(W*g1) (exact because sum_c (x-m) = 0 per pixel).
  q,k are then produced directly from raw x; the per-pixel 1/std enters as
  a weight inv_n = 1/var_n on the pixel-contraction of the S-gram
  (S[c,d] = sum_n inv_n q_cn k_dn) and of the q/k norm sums.  Per-pixel
  stats are computed via ones-matmuls in row layout, round-tripped through
  DRAM into pixel-partition column layout for cheap vector postprocessing.
  The attention output + projection collapses into a per-image 384x384
  matrix G = Wproj @ concat_h(attn_h @ Wv_h) (Wv row-centered on the host, so
  G is automatically column-centered); pass B computes
  y = x + rstd ⊙ (G @ x) with rstd broadcast via ones-column matmuls.
  FFN: LN2 folded into W1'' = W1*g2 - rowmean likewise; yn = (y - m2)*rstd2
  materialized once per chunk in bf16; gelu on scalar engine; ffn2
  interleaved with ffn1 (lag 2) to keep the PE busy.
"""

import numpy as np
import ml_dtypes

B, C, NH, CH, N, HID = 16, 384, 8, 48, 4096, 1536
NCORES = 8
BPC = B // NCORES  # images per core
P = 128
KS = C // P    # 3 k-subtiles for C
KH = HID // P  # 12 k-subtiles for HID
FG = 512       # pixel chunk
NFG = N // FG  # 8
NT = N // P    # 32 128-pixel chunks
LOGIT_MAX = float(np.log(1.0 / 0.01))
EPS_LN = 1e-5
EPS_NORM = 1e-12

_CACHE = {}


def _patch_tile_drain():
    """Walrus in this env rejects >1 sync-wait on the kernel-tail Drain
    (CTRL_NO_STRUCT setupSyncWait).  Split the waits across a chain of
    drain instructions, one wait each.  Idempotent, in-process only."""
    import concourse.tile as tile
    from concourse import mybir
    from concourse.vector_clock import ScopedClock

    if getattr(tile.TileContext._drain_and_barrier, "_split_patch", False):
        return

    def _split_drain(self, tick_clock, wait_clock):
        drain_inst = self.nc.sync.drain()
        wait_clock.add_sem_waits(
            drain_inst.ins, ScopedClock({None: tick_clock.global_clock}))
        si = drain_inst.ins.sync_info
        if si is not None and si.on_wait and len(si.on_wait) > 1:
            waits = list(si.on_wait)
            si.on_wait = waits[:1]
            for w in waits[1:]:
                d2 = self.nc.sync.drain()
                d2.ins.sync_info = mybir.SyncInfo(on_wait=[w], on_update=[])
        self.nc.all_engine_barrier()
        popped = self.nc._tile_sem_poison_stack.pop()
        assert popped is self._sem_poison
        self.nc.clear_and_free_semaphores(list(self.sems.allocated().values()))
        self.nc.all_engine_barrier()

    _split_drain._split_patch = True
    tile.TileContext._drain_and_barrier = _split_drain


def _split_waits(nc, max_waits=1):
    """This walrus build rejects instructions carrying more than one sync
    wait ('Too many sync wait commands' / 'ISA wrong length').  Move extra
    waits onto same-engine NoOps inserted immediately before."""
    from concourse import mybir

    n = 0
    for fn in nc.m.functions:
        for blk in fn.blocks:
            out = []
            for inst in blk.instructions:
                si = inst.sync_info
                if si is not None and si.on_wait and len(si.on_wait) > max_waits:
                    waits = list(si.on_wait)
                    for w in waits[:-max_waits]:
                        n += 1
                        nop = mybir.InstNoOp(
                            name=f"I-wsplit-{n}", ins=[], outs=[])
                        nop.engine = inst.engine
                        nop.sync_info = mybir.SyncInfo(
                            on_wait=[w], on_update=[])
                        out.append(nop)
                    si.on_wait = waits[-max_waits:]
                out.append(inst)
            blk.instructions = out
    return nc


def _build_nc():
    import concourse.bass as bass
    import concourse.tile as tile
    from concourse import mybir
    from concourse.masks import make_identity

    dt = mybir.dt
    AF = mybir.ActivationFunctionType
    ALU = mybir.AluOpType
    AX = mybir.AxisListType

    f32 = dt.float32
    bf16 = dt.bfloat16
    f8 = dt.float8e4
    DR = mybir.MatmulPerfMode.DoubleRow

    _patch_tile_drain()
    nc = bass.Bass()

    xs = nc.declare_dram_parameter("xs", [BPC, C, N], f32, isOutput=False)
    wqk_d = nc.declare_dram_parameter("wqk", [P, 4, 2 * C], f8, isOutput=False)
    wv_d = nc.declare_dram_parameter("wv", [CH, NH, C], bf16, isOutput=False)
    wpj_d = nc.declare_dram_parameter("wpj", [CH, NH, C], bf16, isOutput=False)
    w1_d = nc.declare_dram_parameter("w1", [P, 4, HID], f8, isOutput=False)
    w2_d = nc.declare_dram_parameter("w2", [P, KH, C], f8, isOutput=False)
    scale_d = nc.declare_dram_parameter("scale_row", [1, NH], f32, isOutput=False)
    out_d = nc.declare_dram_parameter("out", [BPC, C, N], f32, isOutput=True)

    with tile.TileContext(nc) as tc:
        with (
            tc.tile_pool(name="consts", bufs=1) as consts,
            tc.tile_pool(name="resA", bufs=1) as resA,
            tc.tile_pool(name="resB", bufs=1) as resB,
            tc.tile_pool(name="work", bufs=2) as work,
            tc.tile_pool(name="ps", bufs=3, space="PSUM") as ps,
            tc.tile_pool(name="psacc", bufs=1, space="PSUM") as psacc,
            tc.tile_pool(name="dram", bufs=2, space="DRAM") as dramp,
        ):
            def bcast_read(dst, dram_row, parts):
                src = bass.AP(
                    tensor=dram_row.tensor, offset=dram_row.offset,
                    ap=[[0, parts]] + [list(d) for d in dram_row.ap[-1:]])
                nc.gpsimd.dma_start(dst, src)

            # ----------------- constants -----------------
            wqk_sb = consts.tile([P, 4, 2 * C], f8, tag="wqk")
            nc.scalar.dma_start(wqk_sb[:], wqk_d[:])
            wv_sb = consts.tile([CH, NH, C], bf16, tag="wv")
            nc.scalar.dma_start(wv_sb[:], wv_d[:])
            wpj_sb = consts.tile([CH, NH, C], bf16, tag="wpj")
            nc.scalar.dma_start(wpj_sb[:], wpj_d[:])
            w1_sb = consts.tile([P, 4, HID], f8, tag="w1")
            nc.scalar.dma_start(w1_sb[:], w1_d[:])
            w2_sb = consts.tile([P, KH, C], f8, tag="w2")
            nc.scalar.dma_start(w2_sb[:], w2_d[:])
            ones_f = consts.tile([P, 1], f32, tag="onesf")
            nc.vector.memset(ones_f[:], 1.0)
            ones_bf = consts.tile([P, 1], bf16, tag="ones")
            nc.vector.tensor_copy(ones_bf[:], ones_f[:])
            onesrow_f = consts.tile([1, P], f32, tag="onesrowf")
            nc.vector.memset(onesrow_f[:], 1.0)
            onesrow_bf = consts.tile([1, P], bf16, tag="onesrow")
            nc.vector.tensor_copy(onesrow_bf[:], onesrow_f[:])
            ident_bf = consts.tile([P, P], bf16, tag="ident")
            make_identity(nc, ident_bf[:])
            ones_f8 = consts.tile([P, 1], f8, tag="ones8")
            nc.vector.tensor_copy(ones_f8[:], ones_f[:])
            schb = consts.tile([CH, NH], f32, tag="schb")
            bcast_read(schb[:], scale_d[0, :], parts=CH)

            xs_r = xs.rearrange("b (s p) n -> b p s n", p=P)
            out_r = out_d.rearrange("b (s p) n -> b p s n", p=P)

            for img in range(BPC):
                st_dram = dramp.tile([2, N], f32, tag="st")
                st2_dram = dramp.tile([2, N], f32, tag="st2")
                nq_dram = dramp.tile([1, C], f32, tag="nq")

                xbf = resA.tile([P, KS, N], bf16, tag="xbf", bufs=1)
                invcol = resA.tile([P, NT], f32, tag="invc", bufs=2)
                ps_s = psacc.tile([CH, NH, CH], f32, tag="S")
                norms = psacc.tile([33, C], f32, tag="N")

                # ---------------- pass A: stats + qk + S/norm accum ------
                for f in range(NFG):
                    sl = slice(f * FG, (f + 1) * FG)
                    xc = work.tile([P, KS, FG], f32, tag="xcf")
                    nc.sync.dma_start(xc[:], xs_r[img][:, :, sl])
                    nc.vector.tensor_copy(xbf[:, :, sl], xc[:])
                    x8 = work.tile([P, 4, FG], f8, tag="x8")
                    nc.gpsimd.memset(x8[:, 3, :], 0.0)
                    nc.vector.tensor_copy(x8[:, 0:KS, :], xc[:])
                    xsq = work.tile([P, 4, FG], f8, tag="xsq", bufs=1)
                    nc.gpsimd.tensor_mul(xsq[:], x8[:], x8[:])
                    pst_a = ps.tile([1, FG], f32, tag="ps")
                    pst_b = ps.tile([1, FG], f32, tag="ps")
                    for s in range(KS):
                        nc.tensor.matmul(
                            pst_a[:], ones_f8[:], x8[:, s, :],
                            start=(s == 0), stop=(s == KS - 1))
                    for s in range(KS):
                        nc.tensor.matmul(
                            pst_b[:], ones_f8[:], xsq[:, s, :],
                            start=(s == 0), stop=(s == KS - 1))
                    srow = work.tile([1, 2, FG], f32, tag="srow")
                    nc.vector.tensor_copy(srow[0:1, 0, :], pst_a[:])
                    nc.vector.tensor_copy(srow[0:1, 1, :], pst_b[:])
                    nc.sync.dma_start(st_dram[:, sl], srow[:])
                    cstat = work.tile([P, 2, 4], f32, tag="cst")
                    for kk in range(2):
                        nc.gpsimd.dma_start(
                            cstat[:, kk, :],
                            st_dram[kk, sl].rearrange("(j p) -> p j", p=P))
                    mcol = work.tile([P, 4], f32, tag="mcol")
                    nc.vector.tensor_scalar(
                        mcol[:], cstat[:, 0, :], 1.0 / C, None, op0=ALU.mult)
                    vcol = work.tile([P, 4], f32, tag="vcol")
                    nc.vector.tensor_scalar(
                        vcol[:], cstat[:, 1, :], 1.0 / C, EPS_LN,
                        op0=ALU.mult, op1=ALU.add)
                    nc.vector.tensor_mul(mcol[:], mcol[:], mcol[:])
                    nc.vector.tensor_sub(vcol[:], vcol[:], mcol[:])
                    c4 = slice(4 * f, 4 * f + 4)
                    nc.vector.reciprocal(invcol[:, c4], vcol[:])

                    qsc8 = work.tile([P, 2, C], f8, tag="qsc8", bufs=2)
                    qkk8 = work.tile([P, 2, C], f8, tag="qkk8", bufs=2)
                    for t in range(4):
                        j = 4 * f + t
                        par = t % 2
                        pa = ps.tile([P, 512], f32, tag="ps")
                        pb = ps.tile([P, 256], f32, tag="ps")
                        lsl = slice(t * P, (t + 1) * P)
                        for pr in range(2):
                            pp2 = slice(2 * pr, 2 * pr + 2)
                            nc.tensor.matmul(
                                pa[:], x8[:, pp2, lsl], wqk_sb[:, pp2, 0:512],
                                start=(pr == 0), stop=(pr == 1), perf_mode=DR)
                            nc.tensor.matmul(
                                pb[:], x8[:, pp2, lsl], wqk_sb[:, pp2, 512:768],
                                start=(pr == 0), stop=(pr == 1), perf_mode=DR)
                        # qsc8 = q*inv ; qkk8 = k ; sqw = 16*inv*q^2 | 16*k^2
                        nc.vector.tensor_scalar(
                            qsc8[:, par, :], pa[:, 0:C], invcol[:, j:j + 1],
                            1.0 / 16.0, op0=ALU.mult, op1=ALU.mult)
                        nc.vector.tensor_scalar(
                            qkk8[:, par, 0:P], pa[:, C:512], 1.0 / 16.0,
                            None, op0=ALU.mult)
                        nc.vector.tensor_scalar(
                            qkk8[:, par, P:C], pb[:], 1.0 / 16.0,
                            None, op0=ALU.mult)
                        sqw = work.tile([P, 2 * C], bf16, tag="sqw", bufs=2)
                        nc.vector.tensor_mul(
                            sqw[:, 0:C], qsc8[:, par, :], pa[:, 0:C])
                        nc.vector.tensor_mul(
                            sqw[:, C:C + P], qkk8[:, par, 0:P], pa[:, C:512])
                        nc.vector.tensor_mul(
                            sqw[:, C + P:2 * C], qkk8[:, par, P:C], pb[:])
                        st_, sp_ = (j == 0), (j == NT - 1)
                        nc.tensor.matmul(
                            norms[0:1, :], ones_bf[:], sqw[:, 0:C],
                            start=st_, stop=sp_)
                        nc.tensor.matmul(
                            norms[32:33, :], ones_bf[:], sqw[:, C:2 * C],
                            start=st_, stop=sp_)
                        if par == 1:
                            sS_, sP_ = (j == 1), (j == NT - 1)
                            for h in range(NH):
                                hs48 = slice(h * CH, (h + 1) * CH)
                                nc.tensor.matmul(
                                    ps_s[:, h, :],
                                    qsc8[:, :, hs48],
                                    qkk8[:, :, hs48],
                                    start=sS_, stop=sP_, perf_mode=DR)

                # ---------------- attention + G build --------------------
                nqrow = work.tile([1, C], f32, tag="nqrow", bufs=1)
                nc.vector.tensor_copy(nqrow[:], norms[0:1, :])
                nc.sync.dma_start(nq_dram[:], nqrow[:])
                rqk = work.tile([CH, NH], f32, tag="rqk", bufs=1)
                nc.gpsimd.dma_start(
                    rqk[:], nq_dram.rearrange("a (h d) -> d (a h)", d=CH))
                rkrow = work.tile([1, C], f32, tag="rkrow", bufs=1)
                nc.scalar.activation(rkrow[:], norms[32:33, :], AF.Sqrt)
                nc.vector.tensor_scalar_max(rkrow[:], rkrow[:], EPS_NORM)
                rki = work.tile([1, C], f32, tag="rki", bufs=1)
                nc.vector.reciprocal(rki[:], rkrow[:])
                rk_bf = work.tile([1, C], bf16, tag="rkbf", bufs=1)
                nc.vector.tensor_copy(rk_bf[:], rki[:])
                rkb = ps.tile([CH, C], f32, tag="ps")
                nc.tensor.matmul(
                    rkb[:], onesrow_bf[0:1, 0:CH], rk_bf[:],
                    start=True, stop=True)
                rqc = work.tile([CH, NH], f32, tag="rqc", bufs=1)
                nc.scalar.activation(rqc[:], rqk[:], AF.Sqrt)
                nc.vector.tensor_scalar_max(rqc[:], rqc[:], EPS_NORM)
                rqi = work.tile([CH, NH], f32, tag="rqi", bufs=1)
                nc.vector.reciprocal(rqi[:], rqc[:])
                nc.vector.tensor_mul(rqi[:], rqi[:], schb[:])
                sS = work.tile([CH, NH, CH], f32, tag="sS", bufs=1)
                nc.vector.tensor_mul(
                    sS[:], ps_s[:],
                    rqi[:, :, None].to_broadcast((CH, NH, CH)))
                rkb3 = rkb.rearrange("d (h e) -> d h e", e=CH)
                nc.vector.tensor_mul(sS[:], sS[:], rkb3)
                expS = work.tile([CH, NH, CH], f32, tag="expS", bufs=1)
                nc.scalar.activation(expS[:], sS[:], AF.Exp)
                esum = work.tile([CH, NH, 1], f32, tag="esum", bufs=1)
                nc.vector.reduce_sum(esum[:], expS[:], axis=AX.X)
                esi = work.tile([CH, NH, 1], f32, tag="esi", bufs=1)
                nc.vector.reciprocal(esi[:], esum[:])
                attn_bf = work.tile([CH, NH, CH], bf16, tag="attnb", bufs=1)
                nc.vector.tensor_mul(
                    attn_bf[:], expS[:], esi.to_broadcast((CH, NH, CH)))
                m1 = work.tile([CH, NH, C], bf16, tag="m1", bufs=1)
                for h in range(NH):
                    pm = ps.tile([CH, C], f32, tag="ps")
                    nc.tensor.matmul(
                        pm[:], attn_bf[:, h, :], wpj_sb[:, h, :],
                        start=True, stop=True)
                    nc.vector.tensor_copy(m1[:, h, :], pm[:])
                gbf = resA.tile([P, 4, C], f8, tag="gbf", bufs=2)
                nc.gpsimd.memset(gbf[:, 3, :], 0.0)
                for jc in range(KS):
                    pg = ps.tile([P, C], f32, tag="ps")
                    for h in range(NH):
                        nc.tensor.matmul(
                            pg[:], wv_sb[:, h, jc * P:(jc + 1) * P],
                            m1[:, h, :], start=(h == 0), stop=(h == NH - 1))
                    nc.vector.tensor_scalar(
                        gbf[:, jc, :], pg[:], 64.0, None, op0=ALU.mult)
                rstdc = work.tile([P, NT], bf16, tag="rstdc", bufs=1)
                nc.scalar.activation(rstdc[:], invcol[:], AF.Sqrt)
                psT = ps.tile([NT, P], bf16, tag="ps")
                nc.tensor.transpose(psT[:], rstdc[:], ident_bf[:])
                rstdT = work.tile([NT, P], bf16, tag="rstdT", bufs=1)
                nc.vector.tensor_copy(rstdT[:], psT[:])
                rstd_row = resA.tile([1, NT, P], bf16, tag="rstdrow", bufs=2)
                nc.gpsimd.dma_start(rstd_row[:], rstdT[:])

                # ---------------- pass B1: y = x + attn branch + stats ---
                ybf = resB.tile([P, KS, N], bf16, tag="ybf")
                for f in range(NFG):
                    sl = slice(f * FG, (f + 1) * FG)
                    psR = ps.tile([P, FG], f32, tag="ps")
                    nc.tensor.matmul(
                        psR[:], onesrow_bf[:],
                        rstd_row.rearrange("a j p -> a (j p)")[:, sl],
                        start=True, stop=True)
                    rb_sb = work.tile([P, FG], bf16, tag="rbsb", bufs=1)
                    nc.vector.tensor_scalar(
                        rb_sb[:], psR[:], 1.0 / 64.0, None, op0=ALU.mult)
                    x8b = work.tile([P, 4, FG], f8, tag="x8")
                    nc.gpsimd.memset(x8b[:, 3, :], 0.0)
                    nc.vector.tensor_copy(x8b[:, 0:KS, :], xbf[:, :, sl])
                    for jc in range(KS):
                        px = ps.tile([P, FG], f32, tag="ps")
                        for pr in range(2):
                            pp2 = slice(2 * pr, 2 * pr + 2)
                            nc.tensor.matmul(
                                px[:], gbf[:, pp2, jc * P:(jc + 1) * P],
                                x8b[:, pp2, :],
                                start=(pr == 0), stop=(pr == 1), perf_mode=DR)
                        nc.vector.tensor_mul(ybf[:, jc, sl], px[:], rb_sb[:])
                        nc.vector.tensor_add(
                            ybf[:, jc, sl], ybf[:, jc, sl], xbf[:, jc, sl])
                    ysq = work.tile([P, KS, FG], bf16, tag="ysq")
                    nc.gpsimd.tensor_mul(ysq[:], ybf[:, :, sl], ybf[:, :, sl])
                    pst_a = ps.tile([1, FG], f32, tag="ps")
                    pst_b = ps.tile([1, FG], f32, tag="ps")
                    for s in range(KS):
                        nc.tensor.matmul(
                            pst_a[:], ones_bf[:], ybf[:, s, sl],
                            start=(s == 0), stop=(s == KS - 1))
                    for s in range(KS):
                        nc.tensor.matmul(
                            pst_b[:], ones_bf[:], ysq[:, s, :],
                            start=(s == 0), stop=(s == KS - 1))
                    srow2 = work.tile([1, 2, FG], f32, tag="srow")
                    nc.vector.tensor_copy(srow2[0:1, 0, :], pst_a[:])
                    nc.vector.tensor_copy(srow2[0:1, 1, :], pst_b[:])
                    nc.sync.dma_start(st2_dram[:, sl], srow2[:])
                cst2 = work.tile([P, 2, NT], f32, tag="cst2", bufs=1)
                for kk in range(2):
                    nc.gpsimd.dma_start(
                        cst2[:, kk, :],
                        st2_dram[kk, :].rearrange("(j p) -> p j", p=P))
                mr2 = work.tile([P, 2, NT], f32, tag="mr2", bufs=1)
                nc.vector.tensor_scalar(
                    mr2[:, 0, :], cst2[:, 0, :], -1.0 / C, None, op0=ALU.mult)
                v2 = work.tile([P, NT], f32, tag="v2", bufs=1)
                nc.vector.tensor_scalar(
                    v2[:], cst2[:, 1, :], 1.0 / C, EPS_LN,
                    op0=ALU.mult, op1=ALU.add)
                msq2 = work.tile([P, NT], f32, tag="msq2", bufs=1)
                nc.vector.tensor_mul(msq2[:], mr2[:, 0, :], mr2[:, 0, :])
                nc.vector.tensor_sub(v2[:], v2[:], msq2[:])
                vi2 = work.tile([P, NT], f32, tag="vi2", bufs=1)
                nc.vector.reciprocal(vi2[:], v2[:])
                nc.scalar.activation(mr2[:, 1, :], vi2[:], AF.Sqrt, scale=256.0)
                nc.vector.tensor_mul(mr2[:, 0, :], mr2[:, 0, :], mr2[:, 1, :])
                mr2_bf = work.tile([P, 2, NT], bf16, tag="mr2b", bufs=1)
                nc.vector.tensor_copy(mr2_bf[:], mr2[:])
                psT2 = ps.tile([2 * NT, P], bf16, tag="ps")
                nc.tensor.transpose(
                    psT2[:], mr2_bf.rearrange("p two j -> p (two j)"),
                    ident_bf[:])
                m2T = work.tile([2 * NT, P], bf16, tag="m2T", bufs=1)
                nc.vector.tensor_copy(m2T[:], psT2[:])
                m2_row = resB.tile([1, 2, NT, P], bf16, tag="m2row", bufs=1)
                nc.gpsimd.dma_start(m2_row[:], m2T[:])

                # ---------------- pass B2: LN2 + FFN + residual ----------
                for f in range(NFG):
                    sl = slice(f * FG, (f + 1) * FG)
                    bcM = ps.tile([P, FG], f32, tag="ps")
                    bcR = ps.tile([P, FG], f32, tag="ps")
                    m2f = m2_row.rearrange("a two j p -> a two (j p)")
                    nc.tensor.matmul(
                        bcM[:], onesrow_bf[:], m2f[:, 0, sl],
                        start=True, stop=True)
                    nc.tensor.matmul(
                        bcR[:], onesrow_bf[:], m2f[:, 1, sl],
                        start=True, stop=True)
                    t_yn = work.tile([P, KS, FG], bf16, tag="tyn", bufs=1)
                    nc.vector.tensor_mul(
                        t_yn[:], ybf[:, :, sl],
                        bcR[:, None, :].to_broadcast((P, KS, FG)))
                    yn = work.tile([P, 4, FG], f8, tag="yn")
                    nc.gpsimd.memset(yn[:, 3, :], 0.0)
                    nc.vector.tensor_add(
                        yn[:, 0:KS, :], t_yn[:],
                        bcM[:, None, :].to_broadcast((P, KS, FG)))
                    h_f8 = work.tile([P, KH, FG], f8, tag="h", bufs=1)
                    po_t = [ps.tile([P, FG], f32, tag="po", bufs=3,
                                    name=f"po{o}")
                            for o in range(KS)]

                    def ffn2_pair(j2):
                        for o in range(KS):
                            nc.tensor.matmul(
                                po_t[o][:],
                                w2_sb[:, 2 * j2:2 * j2 + 2, o * P:(o + 1) * P],
                                h_f8[:, 2 * j2:2 * j2 + 2, :],
                                start=(j2 == 0), stop=(j2 == KH // 2 - 1),
                                perf_mode=DR)

                    for m in range(KH):
                        ph = ps.tile([P, FG], f32, tag="ps")
                        for pr in range(2):
                            nc.tensor.matmul(
                                ph[:],
                                w1_sb[:, 2 * pr:2 * pr + 2, m * P:(m + 1) * P],
                                yn[:, 2 * pr:2 * pr + 2, :],
                                start=(pr == 0), stop=(pr == 1),
                                perf_mode=DR)
                        nc.scalar.activation(
                            h_f8[:, m, :], ph[:], AF.Gelu, scale=1.0 / 256.0)
                        if m >= 3 and (m - 3) % 2 == 0:
                            ffn2_pair((m - 3) // 2)
                    ffn2_pair(KH // 2 - 1)
                    out_t = work.tile([P, KS, FG], f32, tag="xcf")
                    for o in range(KS):
                        nc.vector.scalar_tensor_tensor(
                            out_t[:, o, :], po_t[o][:], 1.0 / 16.0,
                            ybf[:, o, sl], op0=ALU.mult, op1=ALU.add)
                    nc.sync.dma_start(out_r[img][:, :, sl], out_t[:])
    return _split_waits(nc)


def _prep_weights(inputs):
    bf = ml_dtypes.bfloat16
    f8 = ml_dtypes.float8_e4m3fn
    w_qkv = np.asarray(inputs["w_qkv"], np.float64)
    g1 = np.asarray(inputs["g1"], np.float64)
    g2 = np.asarray(inputs["g2"], np.float64)
    for name in ("beta1", "beta2", "b_qkv", "b_proj", "b_ffn1", "b_ffn2"):
        assert not np.any(np.asarray(inputs[name])), f"{name} nonzero unsupported"
    wg = w_qkv * g1[None, :]
    wg = wg - wg.mean(axis=1, keepdims=True)  # fold LN mean-subtraction
    wg3 = wg.reshape(NH, 3 * CH, C)
    wq = wg3[:, 0:CH, :]
    wk = wg3[:, CH:2 * CH, :]
    wv_ = wg3[:, 2 * CH:3 * CH, :]
    # qk columns: all q heads first (384), then all k heads (384)
    wqk = np.concatenate(
        [wq.reshape(C, C), wk.reshape(C, C)], axis=0)  # [768, 384]
    wqk_r = np.zeros((P, 4, 2 * C), np.float64)  # K padded 384 -> 512
    wqk_r[:, 0:KS, :] = (16.0 * wqk).T.reshape(KS, P, 2 * C).transpose(1, 0, 2)
    wv_t = np.ascontiguousarray(wv_.transpose(1, 0, 2))  # [48, NH, 384]
    wpj = np.ascontiguousarray(
        np.asarray(inputs["w_proj"], np.float64).T.reshape(NH, CH, C)
        .transpose(1, 0, 2))  # [d, h, o]
    w1g = np.asarray(inputs["w_ffn1"], np.float64) * g2[None, :]
    w1g = w1g - w1g.mean(axis=1, keepdims=True)
    w1_r = np.zeros((P, 4, HID), np.float64)  # K padded 384 -> 512
    w1_r[:, 0:KS, :] = (16.0 * w1g).T.reshape(KS, P, HID).transpose(1, 0, 2)
    w2_r = np.ascontiguousarray(
        16.0 * np.asarray(inputs["w_ffn2"], np.float64).T
        .reshape(KH, P, C).transpose(1, 0, 2))  # [128, 12, 384]
    ls = np.asarray(inputs["logit_scale"], np.float32).reshape(NH)
    # x16: compensates the 16x scale left in the q/k norm sums (sqw = 16*inv*q^2)
    scale_row = 16.0 * np.exp(np.minimum(ls, LOGIT_MAX))[None, :]
    return dict(
        wqk=np.ascontiguousarray(wqk_r).astype(f8),
        wv=wv_t.astype(bf), wpj=wpj.astype(bf),
        w1=np.ascontiguousarray(w1_r).astype(f8), w2=w2_r.astype(f8),
        scale_row=np.ascontiguousarray(scale_row.astype(np.float32)))


def kernel(**inputs):
    from concourse.bass_utils import run_bass_kernel_spmd

    if "nc" not in _CACHE:
        _CACHE["nc"] = _build_nc()
    nc = _CACHE["nc"]

    x = np.asarray(inputs["x"], np.float32).reshape(B, C, N)
    wmap = _prep_weights(inputs)
    in_maps = []
    for c in range(NCORES):
        m = dict(wmap)
        m["xs"] = np.ascontiguousarray(x[c * BPC:(c + 1) * BPC])
        in_maps.append(m)
    res = run_bass_kernel_spmd(nc, in_maps, list(range(NCORES)))
    out = np.concatenate([r["out"] for r in res.results], axis=0)
    return out.reshape(B, C, 64, 64).astype(np.float32)


# revision 28
# speedup vs baseline: 1.0586x; 1.0586x over previous
"""Trainium2 Bass kernel for nn_CATransformer1 (XCiT-style channel-attention block).

v2: bf16 matmuls, LN centering folded into host-prepared weights, S-gram
weighted by inv-variance on the q side, transpose-free G build, fused
ffn1/ffn2 pipeline with F=512 moving tiles.

Sharding: data-parallel over batch. 16 images / 8 cores = 2 images per core.

Math (per image, x [C=384, N=4096]):
  LN1 gamma and the mean-subtraction are folded into the QKV weights on the
  host: W' = W*g1 - rowmean(W*g1) (exact because sum_c (x-m) = 0 per pixel).
  q,k are then produced directly from raw x; the per-pixel 1/std enters as
  a weight inv_n = 1/var_n on the pixel-contraction of the S-gram
  (S[c,d] = sum_n inv_n q_cn k_dn) and of the q/k norm sums.  Per-pixel
  stats are computed via ones-matmuls in row layout, round-tripped through
  DRAM into pixel-partition column layout for cheap vector postprocessing.
  The attention output + projection collapses into a per-image 384x384
  matrix G = Wproj @ concat_h(attn_h @ Wv_h) (Wv row-centered on the host, so
  G is automatically column-centered); pass B computes
  y = x + rstd ⊙ (G @ x) with rstd broadcast via ones-column matmuls.
  FFN: LN2 folded into W1'' = W1*g2 - rowmean likewise; yn = (y - m2)*rstd2
  materialized once per chunk in bf16; gelu on scalar engine; ffn2
  interleaved with ffn1 (lag 2) to keep the PE busy.
"""

import numpy as np
import ml_dtypes

B, C, NH, CH, N, HID = 16, 384, 8, 48, 4096, 1536
NCORES = 8
BPC = B // NCORES  # images per core
P = 128
KS = C // P    # 3 k-subtiles for C
KH = HID // P  # 12 k-subtiles for HID
FG = 512       # pixel chunk
NFG = N // FG  # 8
NT = N // P    # 32 128-pixel chunks
LOGIT_MAX = float(np.log(1.0 / 0.01))
EPS_LN = 1e-5
EPS_NORM = 1e-12

_CACHE = {}


def _patch_tile_drain():
    """Walrus in this env rejects >1 sync-wait on the kernel-tail Drain
    (CTRL_NO_STRUCT setupSyncWait).  Split the waits across a chain of
    drain instructions, one wait each.  Idempotent, in-process only."""
    import concourse.tile as tile
    from concourse import mybir
    from concourse.vector_clock import ScopedClock

    if getattr(tile.TileContext._drain_and_barrier, "_split_patch", False):
        return

    def _split_drain(self, tick_clock, wait_clock):
        drain_inst = self.nc.sync.drain()
        wait_clock.add_sem_waits(
            drain_inst.ins, ScopedClock({None: tick_clock.global_clock}))
        si = drain_inst.ins.sync_info
        if si is not None and si.on_wait and len(si.on_wait) > 1:
            waits = list(si.on_wait)
            si.on_wait = waits[:1]
            for w in waits[1:]:
                d2 = self.nc.sync.drain()
                d2.ins.sync_info = mybir.SyncInfo(on_wait=[w], on_update=[])
        self.nc.all_engine_barrier()
        popped = self.nc._tile_sem_poison_stack.pop()
        assert popped is self._sem_poison
        self.nc.clear_and_free_semaphores(list(self.sems.allocated().values()))
        self.nc.all_engine_barrier()

    _split_drain._split_patch = True
    tile.TileContext._drain_and_barrier = _split_drain


def _split_waits(nc, max_waits=1):
    """This walrus build rejects instructions carrying more than one sync
    wait ('Too many sync wait commands' / 'ISA wrong length').  Move extra
    waits onto same-engine NoOps inserted immediately before."""
    from concourse import mybir

    n = 0
    for fn in nc.m.functions:
        for blk in fn.blocks:
            out = []
            for inst in blk.instructions:
                si = inst.sync_info
                if si is not None and si.on_wait and len(si.on_wait) > max_waits:
                    waits = list(si.on_wait)
                    for w in waits[:-max_waits]:
                        n += 1
                        nop = mybir.InstNoOp(
                            name=f"I-wsplit-{n}", ins=[], outs=[])
                        nop.engine = inst.engine
                        nop.sync_info = mybir.SyncInfo(
                            on_wait=[w], on_update=[])
                        out.append(nop)
                    si.on_wait = waits[-max_waits:]
                out.append(inst)
            blk.instructions = out
    return nc


def _build_nc():
    import concourse.bass as bass
    import concourse.tile as tile
    from concourse import mybir
    from concourse.masks import make_identity

    dt = mybir.dt
    AF = mybir.ActivationFunctionType
    ALU = mybir.AluOpType
    AX = mybir.AxisListType

    f32 = dt.float32
    bf16 = dt.bfloat16
    f8 = dt.float8e4
    DR = mybir.MatmulPerfMode.DoubleRow

    _patch_tile_drain()
    nc = bass.Bass()

    xs = nc.declare_dram_parameter("xs", [BPC, C, N], f32, isOutput=False)
    wqk_d = nc.declare_dram_parameter("wqk", [P, 4, 2 * C], f8, isOutput=False)
    wv_d = nc.declare_dram_parameter("wv", [CH, NH, C], bf16, isOutput=False)
    wpj_d = nc.declare_dram_parameter("wpj", [CH, NH, C], bf16, isOutput=False)
    w1_d = nc.declare_dram_parameter("w1", [P, 4, HID], f8, isOutput=False)
    w2_d = nc.declare_dram_parameter("w2", [P, KH, C], f8, isOutput=False)
    scale_d = nc.declare_dram_parameter("scale_row", [1, NH], f32, isOutput=False)
    out_d = nc.declare_dram_parameter("out", [BPC, C, N], f32, isOutput=True)

    with tile.TileContext(nc) as tc:
        with (
            tc.tile_pool(name="consts", bufs=1) as consts,
            tc.tile_pool(name="resA", bufs=1) as resA,
            tc.tile_pool(name="resB", bufs=1) as resB,
            tc.tile_pool(name="work", bufs=2) as work,
            tc.tile_pool(name="ps", bufs=4, space="PSUM") as ps,
            tc.tile_pool(name="psacc", bufs=1, space="PSUM") as psacc,
            tc.tile_pool(name="dram", bufs=2, space="DRAM") as dramp,
        ):
            def bcast_read(dst, dram_row, parts):
                src = bass.AP(
                    tensor=dram_row.tensor, offset=dram_row.offset,
                    ap=[[0, parts]] + [list(d) for d in dram_row.ap[-1:]])
                nc.gpsimd.dma_start(dst, src)

            # ----------------- constants -----------------
            wqk_sb = consts.tile([P, 4, 2 * C], f8, tag="wqk")
            nc.scalar.dma_start(wqk_sb[:], wqk_d[:])
            wv_sb = consts.tile([CH, NH, C], bf16, tag="wv")
            nc.scalar.dma_start(wv_sb[:], wv_d[:])
            wpj_sb = consts.tile([CH, NH, C], bf16, tag="wpj")
            nc.scalar.dma_start(wpj_sb[:], wpj_d[:])
            w1_sb = consts.tile([P, 4, HID], f8, tag="w1")
            nc.scalar.dma_start(w1_sb[:], w1_d[:])
            w2_sb = consts.tile([P, KH, C], f8, tag="w2")
            nc.scalar.dma_start(w2_sb[:], w2_d[:])
            ones_f = consts.tile([P, 1], f32, tag="onesf")
            nc.vector.memset(ones_f[:], 1.0)
            ones_bf = consts.tile([P, 1], bf16, tag="ones")
            nc.vector.tensor_copy(ones_bf[:], ones_f[:])
            onesrow_f = consts.tile([1, P], f32, tag="onesrowf")
            nc.vector.memset(onesrow_f[:], 1.0)
            onesrow_bf = consts.tile([1, P], bf16, tag="onesrow")
            nc.vector.tensor_copy(onesrow_bf[:], onesrow_f[:])
            ident_bf = consts.tile([P, P], bf16, tag="ident")
            make_identity(nc, ident_bf[:])
            ones_f8 = consts.tile([P, 1], f8, tag="ones8")
            nc.vector.tensor_copy(ones_f8[:], ones_f[:])
            schb = consts.tile([CH, NH], f32, tag="schb")
            bcast_read(schb[:], scale_d[0, :], parts=CH)

            xs_r = xs.rearrange("b (s p) n -> b p s n", p=P)
            out_r = out_d.rearrange("b (s p) n -> b p s n", p=P)

            for img in range(BPC):
                st_dram = dramp.tile([2, N], f32, tag="st")
                st2_dram = dramp.tile([2, N], f32, tag="st2")
                nq_dram = dramp.tile([1, C], f32, tag="nq")

                xbf = resA.tile([P, KS, N], bf16, tag="xbf", bufs=1)
                invcol = resA.tile([P, NT], f32, tag="invc", bufs=2)
                ps_s = psacc.tile([CH, NH, CH], f32, tag="S")
                norms = psacc.tile([33, C], f32, tag="N")
                nq_ps = norms[0:1, :]
                nk_ps = norms[32:33, :]

                # ---------------- pass A: stats + qk + S/norm accum ------
                for f in range(NFG):
                    sl = slice(f * FG, (f + 1) * FG)
                    xc = work.tile([P, KS, FG], f32, tag="xcf")
                    nc.sync.dma_start(xc[:], xs_r[img][:, :, sl])
                    nc.vector.tensor_copy(xbf[:, :, sl], xc[:])
                    x8 = work.tile([P, 4, FG], f8, tag="x8")
                    nc.gpsimd.memset(x8[:, 3, :], 0.0)
                    nc.vector.tensor_copy(x8[:, 0:KS, :], xc[:])
                    xsq = work.tile([P, 4, FG], f8, tag="xsq", bufs=2)
                    nc.gpsimd.tensor_mul(xsq[:], x8[:], x8[:])
                    pst_a = ps.tile([1, FG], f32, tag="ps")
                    pst_b = ps.tile([1, FG], f32, tag="ps")
                    for s in range(KS):
                        nc.tensor.matmul(
                            pst_a[:], ones_f8[:], x8[:, s, :],
                            start=(s == 0), stop=(s == KS - 1))
                    for s in range(KS):
                        nc.tensor.matmul(
                            pst_b[:], ones_f8[:], xsq[:, s, :],
                            start=(s == 0), stop=(s == KS - 1))
                    srow = work.tile([1, 2, FG], f32, tag="srow")
                    nc.vector.tensor_copy(srow[0:1, 0, :], pst_a[:])
                    nc.vector.tensor_copy(srow[0:1, 1, :], pst_b[:])
                    nc.sync.dma_start(st_dram[:, sl], srow[:])
                    cstat = work.tile([P, 2, 4], f32, tag="cst")
                    for kk in range(2):
                        nc.gpsimd.dma_start(
                            cstat[:, kk, :],
                            st_dram[kk, sl].rearrange("(j p) -> p j", p=P))
                    mcol = work.tile([P, 4], f32, tag="mcol")
                    nc.vector.tensor_scalar(
                        mcol[:], cstat[:, 0, :], 1.0 / C, None, op0=ALU.mult)
                    vcol = work.tile([P, 4], f32, tag="vcol")
                    nc.vector.tensor_scalar(
                        vcol[:], cstat[:, 1, :], 1.0 / C, EPS_LN,
                        op0=ALU.mult, op1=ALU.add)
                    nc.vector.tensor_mul(mcol[:], mcol[:], mcol[:])
                    nc.vector.tensor_sub(vcol[:], vcol[:], mcol[:])
                    c4 = slice(4 * f, 4 * f + 4)
                    nc.vector.reciprocal(invcol[:, c4], vcol[:])

                    qsc8 = work.tile([P, 2, C], f8, tag="qsc8", bufs=2)
                    qkk8 = work.tile([P, 2, C], f8, tag="qkk8", bufs=2)
                    for t in range(4):
                        j = 4 * f + t
                        par = t % 2
                        pa = ps.tile([P, 512], f32, tag="ps")
                        pb = ps.tile([P, 256], f32, tag="ps")
                        lsl = slice(t * P, (t + 1) * P)
                        for pr in range(2):
                            pp2 = slice(2 * pr, 2 * pr + 2)
                            nc.tensor.matmul(
                                pa[:], x8[:, pp2, lsl], wqk_sb[:, pp2, 0:512],
                                start=(pr == 0), stop=(pr == 1), perf_mode=DR)
                            nc.tensor.matmul(
                                pb[:], x8[:, pp2, lsl], wqk_sb[:, pp2, 512:768],
                                start=(pr == 0), stop=(pr == 1), perf_mode=DR)
                        # evict plain q8 = q, k8 = k (frees PSUM fast), then
                        # qsc8 = q*inv, sqw = inv*q^2 | k^2 from SBUF only
                        q8 = work.tile([P, C], f8, tag="q8", bufs=2)
                        k8 = work.tile([P, C], f8, tag="k8", bufs=2)
                        nc.vector.tensor_scalar(
                            q8[:], pa[:, 0:C], 1.0 / 16.0, None, op0=ALU.mult)
                        nc.vector.tensor_scalar(
                            k8[:, 0:P], pa[:, C:512], 1.0 / 16.0,
                            None, op0=ALU.mult)
                        nc.vector.tensor_scalar(
                            k8[:, P:C], pb[:], 1.0 / 16.0, None, op0=ALU.mult)
                        nc.vector.tensor_scalar_mul(
                            qsc8[:, par, :], q8[:], invcol[:, j:j + 1])
                        nc.vector.tensor_copy(qkk8[:, par, :], k8[:])
                        sqw = work.tile([P, 2 * C], bf16, tag="sqw", bufs=2)
                        nc.vector.tensor_mul(
                            sqw[:, 0:C], qsc8[:, par, :], q8[:])
                        nc.vector.tensor_mul(sqw[:, C:2 * C], k8[:], k8[:])
                        st_, sp_ = (j == 0), (j == NT - 1)
                        nc.tensor.matmul(
                            nq_ps, ones_bf[:], sqw[:, 0:C],
                            start=st_, stop=sp_)
                        nc.tensor.matmul(
                            nk_ps, ones_bf[:], sqw[:, C:2 * C],
                            start=st_, stop=sp_)
                        if par == 1:
                            sS_, sP_ = (j == 1), (j == NT - 1)
                            for h in range(NH):
                                hs48 = slice(h * CH, (h + 1) * CH)
                                nc.tensor.matmul(
                                    ps_s[:, h, :],
                                    qsc8[:, :, hs48],
                                    qkk8[:, :, hs48],
                                    start=sS_, stop=sP_, perf_mode=DR)

                # ---------------- attention + G build --------------------
                nqrow = work.tile([1, C], f32, tag="nqrow", bufs=1)
                nc.vector.tensor_copy(nqrow[:], nq_ps)
                nc.sync.dma_start(nq_dram[:], nqrow[:])
                rqk = work.tile([CH, NH], f32, tag="rqk", bufs=1)
                nc.gpsimd.dma_start(
                    rqk[:], nq_dram.rearrange("a (h d) -> d (a h)", d=CH))
                rkrow = work.tile([1, C], f32, tag="rkrow", bufs=1)
                nc.scalar.activation(rkrow[:], nk_ps, AF.Sqrt)
                nc.vector.tensor_scalar_max(rkrow[:], rkrow[:], EPS_NORM)
                rki = work.tile([1, C], f32, tag="rki", bufs=1)
                nc.vector.reciprocal(rki[:], rkrow[:])
                rk_bf = work.tile([1, C], bf16, tag="rkbf", bufs=1)
                nc.vector.tensor_copy(rk_bf[:], rki[:])
                rkb = ps.tile([CH, C], f32, tag="ps")
                nc.tensor.matmul(
                    rkb[:], onesrow_bf[0:1, 0:CH], rk_bf[:],
                    start=True, stop=True)
                rqc = work.tile([CH, NH], f32, tag="rqc", bufs=1)
                nc.scalar.activation(rqc[:], rqk[:], AF.Sqrt)
                nc.vector.tensor_scalar_max(rqc[:], rqc[:], EPS_NORM)
                rqi = work.tile([CH, NH], f32, tag="rqi", bufs=1)
                nc.vector.reciprocal(rqi[:], rqc[:])
                nc.vector.tensor_mul(rqi[:], rqi[:], schb[:])
                sS = work.tile([CH, NH, CH], f32, tag="sS", bufs=1)
                nc.vector.tensor_mul(
                    sS[:], ps_s[:],
                    rqi[:, :, None].to_broadcast((CH, NH, CH)))
                rkb3 = rkb.rearrange("d (h e) -> d h e", e=CH)
                nc.vector.tensor_mul(sS[:], sS[:], rkb3)
                expS = work.tile([CH, NH, CH], f32, tag="expS", bufs=1)
                nc.scalar.activation(expS[:], sS[:], AF.Exp)
                esum = work.tile([CH, NH, 1], f32, tag="esum", bufs=1)
                nc.vector.reduce_sum(esum[:], expS[:], axis=AX.X)
                esi = work.tile([CH, NH, 1], f32, tag="esi", bufs=1)
                nc.vector.reciprocal(esi[:], esum[:])
                attn_bf = work.tile([CH, NH, CH], bf16, tag="attnb", bufs=1)
                nc.vector.tensor_mul(
                    attn_bf[:], expS[:], esi.to_broadcast((CH, NH, CH)))
                m1 = work.tile([CH, NH, C], bf16, tag="m1", bufs=1)
                for h in range(NH):
                    pm = ps.tile([CH, C], f32, tag="ps")
                    nc.tensor.matmul(
                        pm[:], attn_bf[:, h, :], wpj_sb[:, h, :],
                        start=True, stop=True)
                    nc.vector.tensor_copy(m1[:, h, :], pm[:])
                gbf = resA.tile([P, 4, C], f8, tag="gbf", bufs=2)
                nc.gpsimd.memset(gbf[:, 3, :], 0.0)
                for jc in range(KS):
                    pg = ps.tile([P, C], f32, tag="ps")
                    for h in range(NH):
                        nc.tensor.matmul(
                            pg[:], wv_sb[:, h, jc * P:(jc + 1) * P],
                            m1[:, h, :], start=(h == 0), stop=(h == NH - 1))
                    nc.vector.tensor_scalar(
                        gbf[:, jc, :], pg[:], 64.0, None, op0=ALU.mult)
                rstdc = work.tile([P, NT], bf16, tag="rstdc", bufs=1)
                nc.scalar.activation(rstdc[:], invcol[:], AF.Sqrt)
                psT = ps.tile([NT, P], bf16, tag="ps")
                nc.tensor.transpose(psT[:], rstdc[:], ident_bf[:])
                rstdT = work.tile([NT, P], bf16, tag="rstdT", bufs=1)
                nc.vector.tensor_copy(rstdT[:], psT[:])
                rstd_row = resA.tile([1, NT, P], bf16, tag="rstdrow", bufs=2)
                nc.gpsimd.dma_start(rstd_row[:], rstdT[:])

                # ---------------- pass B1: y = x + attn branch + stats ---
                ybf = resB.tile([P, KS, N], bf16, tag="ybf")
                for f in range(NFG):
                    sl = slice(f * FG, (f + 1) * FG)
                    psR = ps.tile([P, FG], f32, tag="ps")
                    nc.tensor.matmul(
                        psR[:], onesrow_bf[:],
                        rstd_row.rearrange("a j p -> a (j p)")[:, sl],
                        start=True, stop=True)
                    rb_sb = work.tile([P, FG], bf16, tag="rbsb", bufs=1)
                    nc.vector.tensor_scalar(
                        rb_sb[:], psR[:], 1.0 / 64.0, None, op0=ALU.mult)
                    x8b = work.tile([P, 4, FG], f8, tag="x8")
                    nc.gpsimd.memset(x8b[:, 3, :], 0.0)
                    nc.vector.tensor_copy(x8b[:, 0:KS, :], xbf[:, :, sl])
                    for jc in range(KS):
                        px = ps.tile([P, FG], f32, tag="ps")
                        for pr in range(2):
                            pp2 = slice(2 * pr, 2 * pr + 2)
                            nc.tensor.matmul(
                                px[:], gbf[:, pp2, jc * P:(jc + 1) * P],
                                x8b[:, pp2, :],
                                start=(pr == 0), stop=(pr == 1), perf_mode=DR)
                        nc.vector.tensor_mul(ybf[:, jc, sl], px[:], rb_sb[:])
                        nc.vector.tensor_add(
                            ybf[:, jc, sl], ybf[:, jc, sl], xbf[:, jc, sl])
                    ysq = work.tile([P, KS, FG], bf16, tag="ysq")
                    nc.gpsimd.tensor_mul(ysq[:], ybf[:, :, sl], ybf[:, :, sl])
                    pst_a = ps.tile([1, FG], f32, tag="ps")
                    pst_b = ps.tile([1, FG], f32, tag="ps")
                    for s in range(KS):
                        nc.tensor.matmul(
                            pst_a[:], ones_bf[:], ybf[:, s, sl],
                            start=(s == 0), stop=(s == KS - 1))
                    for s in range(KS):
                        nc.tensor.matmul(
                            pst_b[:], ones_bf[:], ysq[:, s, :],
                            start=(s == 0), stop=(s == KS - 1))
                    srow2 = work.tile([1, 2, FG], f32, tag="srow")
                    nc.vector.tensor_copy(srow2[0:1, 0, :], pst_a[:])
                    nc.vector.tensor_copy(srow2[0:1, 1, :], pst_b[:])
                    nc.sync.dma_start(st2_dram[:, sl], srow2[:])
                cst2 = work.tile([P, 2, NT], f32, tag="cst2", bufs=1)
                for kk in range(2):
                    nc.gpsimd.dma_start(
                        cst2[:, kk, :],
                        st2_dram[kk, :].rearrange("(j p) -> p j", p=P))
                mr2 = work.tile([P, 2, NT], f32, tag="mr2", bufs=1)
                nc.vector.tensor_scalar(
                    mr2[:, 0, :], cst2[:, 0, :], -1.0 / C, None, op0=ALU.mult)
                v2 = work.tile([P, NT], f32, tag="v2", bufs=1)
                nc.vector.tensor_scalar(
                    v2[:], cst2[:, 1, :], 1.0 / C, EPS_LN,
                    op0=ALU.mult, op1=ALU.add)
                msq2 = work.tile([P, NT], f32, tag="msq2", bufs=1)
                nc.vector.tensor_mul(msq2[:], mr2[:, 0, :], mr2[:, 0, :])
                nc.vector.tensor_sub(v2[:], v2[:], msq2[:])
                vi2 = work.tile([P, NT], f32, tag="vi2", bufs=1)
                nc.vector.reciprocal(vi2[:], v2[:])
                nc.scalar.activation(mr2[:, 1, :], vi2[:], AF.Sqrt, scale=256.0)
                nc.vector.tensor_mul(mr2[:, 0, :], mr2[:, 0, :], mr2[:, 1, :])
                mr2_bf = work.tile([P, 2, NT], bf16, tag="mr2b", bufs=1)
                nc.vector.tensor_copy(mr2_bf[:], mr2[:])
                psT2 = ps.tile([2 * NT, P], bf16, tag="ps")
                nc.tensor.transpose(
                    psT2[:], mr2_bf.rearrange("p two j -> p (two j)"),
                    ident_bf[:])
                m2T = work.tile([2 * NT, P], bf16, tag="m2T", bufs=1)
                nc.vector.tensor_copy(m2T[:], psT2[:])
                m2_row = resB.tile([1, 2, NT, P], bf16, tag="m2row", bufs=1)
                nc.gpsimd.dma_start(m2_row[:], m2T[:])

                # ---------------- pass B2: LN2 + FFN + residual ----------
                for f in range(NFG):
                    sl = slice(f * FG, (f + 1) * FG)
                    bcM = ps.tile([P, FG], f32, tag="ps")
                    bcR = ps.tile([P, FG], f32, tag="ps")
                    m2f = m2_row.rearrange("a two j p -> a two (j p)")
                    nc.tensor.matmul(
                        bcM[:], onesrow_bf[:], m2f[:, 0, sl],
                        start=True, stop=True)
                    nc.tensor.matmul(
                        bcR[:], onesrow_bf[:], m2f[:, 1, sl],
                        start=True, stop=True)
                    t_yn = work.tile([P, KS, FG], bf16, tag="tyn", bufs=1)
                    nc.vector.tensor_mul(
                        t_yn[:], ybf[:, :, sl],
                        bcR[:, None, :].to_broadcast((P, KS, FG)))
                    yn = work.tile([P, 4, FG], f8, tag="yn")
                    nc.gpsimd.memset(yn[:, 3, :], 0.0)
                    nc.vector.tensor_add(
                        yn[:, 0:KS, :], t_yn[:],
                        bcM[:, None, :].to_broadcast((P, KS, FG)))
                    h_f8 = work.tile([P, KH, FG], f8, tag="h", bufs=1)
                    po_t = [ps.tile([P, FG], f32, tag="po", bufs=2,
                                    name=f"po{o}")
                            for o in range(2)]

                    def ffn2_pair(j2):
                        for o in range(2):
                            nc.tensor.matmul(
                                po_t[o][:],
                                w2_sb[:, 2 * j2:2 * j2 + 2, o * P:(o + 1) * P],
                                h_f8[:, 2 * j2:2 * j2 + 2, :],
                                start=(j2 == 0), stop=(j2 == KH // 2 - 1),
                                perf_mode=DR)

                    for m in range(KH):
                        ph = ps.tile([P, FG], f32, tag="ps")
                        for pr in range(2):
                            nc.tensor.matmul(
                                ph[:],
                                w1_sb[:, 2 * pr:2 * pr + 2, m * P:(m + 1) * P],
                                yn[:, 2 * pr:2 * pr + 2, :],
                                start=(pr == 0), stop=(pr == 1),
                                perf_mode=DR)
                        nc.scalar.activation(
                            h_f8[:, m, :], ph[:], AF.Gelu, scale=1.0 / 256.0)
                        if m >= 3 and (m - 3) % 2 == 0:
                            ffn2_pair((m - 3) // 2)
                    ffn2_pair(KH // 2 - 1)
                    po2 = ps.tile([P, FG], f32, tag="ps")
                    for j2 in range(KH // 2):
                        nc.tensor.matmul(
                            po2[:], w2_sb[:, 2 * j2:2 * j2 + 2, 2 * P:3 * P],
                            h_f8[:, 2 * j2:2 * j2 + 2, :],
                            start=(j2 == 0), stop=(j2 == KH // 2 - 1),
                            perf_mode=DR)
                    out_t = work.tile([P, KS, FG], f32, tag="xcf")
                    for o in range(2):
                        nc.vector.scalar_tensor_tensor(
                            out_t[:, o, :], po_t[o][:], 1.0 / 16.0,
                            ybf[:, o, sl], op0=ALU.mult, op1=ALU.add)
                    nc.vector.scalar_tensor_tensor(
                        out_t[:, 2, :], po2[:], 1.0 / 16.0,
                        ybf[:, 2, sl], op0=ALU.mult, op1=ALU.add)
                    nc.sync.dma_start(out_r[img][:, :, sl], out_t[:])
    return _split_waits(nc)


def _prep_weights(inputs):
    bf = ml_dtypes.bfloat16
    f8 = ml_dtypes.float8_e4m3fn
    w_qkv = np.asarray(inputs["w_qkv"], np.float64)
    g1 = np.asarray(inputs["g1"], np.float64)
    g2 = np.asarray(inputs["g2"], np.float64)
    for name in ("beta1", "beta2", "b_qkv", "b_proj", "b_ffn1", "b_ffn2"):
        assert not np.any(np.asarray(inputs[name])), f"{name} nonzero unsupported"
    wg = w_qkv * g1[None, :]
    wg = wg - wg.mean(axis=1, keepdims=True)  # fold LN mean-subtraction
    wg3 = wg.reshape(NH, 3 * CH, C)
    wq = wg3[:, 0:CH, :]
    wk = wg3[:, CH:2 * CH, :]
    wv_ = wg3[:, 2 * CH:3 * CH, :]
    # qk columns: all q heads first (384), then all k heads (384)
    wqk = np.concatenate(
        [wq.reshape(C, C), wk.reshape(C, C)], axis=0)  # [768, 384]
    wqk_r = np.zeros((P, 4, 2 * C), np.float64)  # K padded 384 -> 512
    wqk_r[:, 0:KS, :] = (16.0 * wqk).T.reshape(KS, P, 2 * C).transpose(1, 0, 2)
    wv_t = np.ascontiguousarray(wv_.transpose(1, 0, 2))  # [48, NH, 384]
    wpj = np.ascontiguousarray(
        np.asarray(inputs["w_proj"], np.float64).T.reshape(NH, CH, C)
        .transpose(1, 0, 2))  # [d, h, o]
    w1g = np.asarray(inputs["w_ffn1"], np.float64) * g2[None, :]
    w1g = w1g - w1g.mean(axis=1, keepdims=True)
    w1_r = np.zeros((P, 4, HID), np.float64)  # K padded 384 -> 512
    w1_r[:, 0:KS, :] = (16.0 * w1g).T.reshape(KS, P, HID).transpose(1, 0, 2)
    w2_r = np.ascontiguousarray(
        16.0 * np.asarray(inputs["w_ffn2"], np.float64).T
        .reshape(KH, P, C).transpose(1, 0, 2))  # [128, 12, 384]
    ls = np.asarray(inputs["logit_scale"], np.float32).reshape(NH)
    scale_row = np.exp(np.minimum(ls, LOGIT_MAX))[None, :]
    return dict(
        wqk=np.ascontiguousarray(wqk_r).astype(f8),
        wv=wv_t.astype(bf), wpj=wpj.astype(bf),
        w1=np.ascontiguousarray(w1_r).astype(f8), w2=w2_r.astype(f8),
        scale_row=np.ascontiguousarray(scale_row.astype(np.float32)))


def kernel(**inputs):
    from concourse.bass_utils import run_bass_kernel_spmd

    if "nc" not in _CACHE:
        _CACHE["nc"] = _build_nc()
    nc = _CACHE["nc"]

    x = np.asarray(inputs["x"], np.float32).reshape(B, C, N)
    wmap = _prep_weights(inputs)
    in_maps = []
    for c in range(NCORES):
        m = dict(wmap)
        m["xs"] = np.ascontiguousarray(x[c * BPC:(c + 1) * BPC])
        in_maps.append(m)
    res = run_bass_kernel_spmd(nc, in_maps, list(range(NCORES)))
    out = np.concatenate([r["out"] for r in res.results], axis=0)
    return out.reshape(B, C, 64, 64).astype(np.float32)


# revision 29
# speedup vs baseline: 1.2202x; 1.1526x over previous
"""Trainium2 Bass kernel for nn_CATransformer1 (XCiT-style channel-attention block).

v2: bf16 matmuls, LN centering folded into host-prepared weights, S-gram
weighted by inv-variance on the q side, transpose-free G build, fused
ffn1/ffn2 pipeline with F=512 moving tiles.

Sharding: data-parallel over batch. 16 images / 8 cores = 2 images per core.

Math (per image, x [C=384, N=4096]):
  LN1 gamma and the mean-subtraction are folded into the QKV weights on the
  host: W' = W*g1 - rowmean(W*g1) (exact because sum_c (x-m) = 0 per pixel).
  q,k are then produced directly from raw x; the per-pixel 1/std enters as
  a weight inv_n = 1/var_n on the pixel-contraction of the S-gram
  (S[c,d] = sum_n inv_n q_cn k_dn) and of the q/k norm sums.  Per-pixel
  stats are computed via ones-matmuls in row layout, round-tripped through
  DRAM into pixel-partition column layout for cheap vector postprocessing.
  The attention output + projection collapses into a per-image 384x384
  matrix G = Wproj @ concat_h(attn_h @ Wv_h) (Wv row-centered on the host, so
  G is automatically column-centered); pass B computes
  y = x + rstd ⊙ (G @ x) with rstd broadcast via ones-column matmuls.
  FFN: LN2 folded into W1'' = W1*g2 - rowmean likewise; yn = (y - m2)*rstd2
  materialized once per chunk in bf16; gelu on scalar engine; ffn2
  interleaved with ffn1 (lag 2) to keep the PE busy.
"""

import numpy as np
import ml_dtypes

B, C, NH, CH, N, HID = 16, 384, 8, 48, 4096, 1536
NCORES = 8
BPC = B // NCORES  # images per core
P = 128
KS = C // P    # 3 k-subtiles for C
KH = HID // P  # 12 k-subtiles for HID
FG = 512       # pixel chunk
NFG = N // FG  # 8
NT = N // P    # 32 128-pixel chunks
LOGIT_MAX = float(np.log(1.0 / 0.01))
EPS_LN = 1e-5
EPS_NORM = 1e-12

_CACHE = {}


def _patch_tile_drain():
    """Walrus in this env rejects >1 sync-wait on the kernel-tail Drain
    (CTRL_NO_STRUCT setupSyncWait).  Split the waits across a chain of
    drain instructions, one wait each.  Idempotent, in-process only."""
    import concourse.tile as tile
    from concourse import mybir
    from concourse.vector_clock import ScopedClock

    if getattr(tile.TileContext._drain_and_barrier, "_split_patch", False):
        return

    def _split_drain(self, tick_clock, wait_clock):
        drain_inst = self.nc.sync.drain()
        wait_clock.add_sem_waits(
            drain_inst.ins, ScopedClock({None: tick_clock.global_clock}))
        si = drain_inst.ins.sync_info
        if si is not None and si.on_wait and len(si.on_wait) > 1:
            waits = list(si.on_wait)
            si.on_wait = waits[:1]
            for w in waits[1:]:
                d2 = self.nc.sync.drain()
                d2.ins.sync_info = mybir.SyncInfo(on_wait=[w], on_update=[])
        self.nc.all_engine_barrier()
        popped = self.nc._tile_sem_poison_stack.pop()
        assert popped is self._sem_poison
        self.nc.clear_and_free_semaphores(list(self.sems.allocated().values()))
        self.nc.all_engine_barrier()

    _split_drain._split_patch = True
    tile.TileContext._drain_and_barrier = _split_drain


def _split_waits(nc, max_waits=1):
    """This walrus build rejects instructions carrying more than one sync
    wait ('Too many sync wait commands' / 'ISA wrong length').  Move extra
    waits onto same-engine NoOps inserted immediately before."""
    from concourse import mybir

    n = 0
    for fn in nc.m.functions:
        for blk in fn.blocks:
            out = []
            for inst in blk.instructions:
                si = inst.sync_info
                if si is not None and si.on_wait and len(si.on_wait) > max_waits:
                    waits = list(si.on_wait)
                    for w in waits[:-max_waits]:
                        n += 1
                        nop = mybir.InstNoOp(
                            name=f"I-wsplit-{n}", ins=[], outs=[])
                        nop.engine = inst.engine
                        nop.sync_info = mybir.SyncInfo(
                            on_wait=[w], on_update=[])
                        out.append(nop)
                    si.on_wait = waits[-max_waits:]
                out.append(inst)
            blk.instructions = out
    return nc


def _build_nc():
    import concourse.bass as bass
    import concourse.tile as tile
    from concourse import mybir
    from concourse.masks import make_identity

    dt = mybir.dt
    AF = mybir.ActivationFunctionType
    ALU = mybir.AluOpType
    AX = mybir.AxisListType

    f32 = dt.float32
    bf16 = dt.bfloat16
    f8 = dt.float8e4
    DR = mybir.MatmulPerfMode.DoubleRow

    _patch_tile_drain()
    nc = bass.Bass()

    xs = nc.declare_dram_parameter("xs", [BPC, C, N], f32, isOutput=False)
    wqk_d = nc.declare_dram_parameter("wqk", [P, 4, 2 * C], f8, isOutput=False)
    wv_d = nc.declare_dram_parameter("wv", [CH, NH, C], bf16, isOutput=False)
    wpj_d = nc.declare_dram_parameter("wpj", [CH, NH, C], bf16, isOutput=False)
    w1_d = nc.declare_dram_parameter("w1", [P, 4, HID], f8, isOutput=False)
    w2_d = nc.declare_dram_parameter("w2", [P, KH, C], f8, isOutput=False)
    scale_d = nc.declare_dram_parameter("scale_row", [1, NH], f32, isOutput=False)
    out_d = nc.declare_dram_parameter("out", [BPC, C, N], f32, isOutput=True)

    with tile.TileContext(nc) as tc:
        with (
            tc.tile_pool(name="consts", bufs=1) as consts,
            tc.tile_pool(name="resA", bufs=1) as resA,
            tc.tile_pool(name="resB", bufs=1) as resB,
            tc.tile_pool(name="work", bufs=2) as work,
            tc.tile_pool(name="ps", bufs=4, space="PSUM") as ps,
            tc.tile_pool(name="psacc", bufs=1, space="PSUM") as psacc,
            tc.tile_pool(name="dram", bufs=2, space="DRAM") as dramp,
        ):
            def bcast_read(dst, dram_row, parts):
                src = bass.AP(
                    tensor=dram_row.tensor, offset=dram_row.offset,
                    ap=[[0, parts]] + [list(d) for d in dram_row.ap[-1:]])
                nc.gpsimd.dma_start(dst, src)

            # ----------------- constants -----------------
            wqk_sb = consts.tile([P, 4, 2 * C], f8, tag="wqk")
            nc.scalar.dma_start(wqk_sb[:], wqk_d[:])
            wv_sb = consts.tile([CH, NH, C], bf16, tag="wv")
            nc.scalar.dma_start(wv_sb[:], wv_d[:])
            wpj_sb = consts.tile([CH, NH, C], bf16, tag="wpj")
            nc.scalar.dma_start(wpj_sb[:], wpj_d[:])
            w1_sb = consts.tile([P, 4, HID], f8, tag="w1")
            nc.scalar.dma_start(w1_sb[:], w1_d[:])
            w2_sb = consts.tile([P, KH, C], f8, tag="w2")
            nc.scalar.dma_start(w2_sb[:], w2_d[:])
            ones_f = consts.tile([P, 1], f32, tag="onesf")
            nc.vector.memset(ones_f[:], 1.0)
            ones_bf = consts.tile([P, 1], bf16, tag="ones")
            nc.vector.tensor_copy(ones_bf[:], ones_f[:])
            onesrow_f = consts.tile([1, P], f32, tag="onesrowf")
            nc.vector.memset(onesrow_f[:], 1.0)
            onesrow_bf = consts.tile([1, P], bf16, tag="onesrow")
            nc.vector.tensor_copy(onesrow_bf[:], onesrow_f[:])
            ident_bf = consts.tile([P, P], bf16, tag="ident")
            make_identity(nc, ident_bf[:])
            ones_f8 = consts.tile([P, 1], f8, tag="ones8")
            nc.vector.tensor_copy(ones_f8[:], ones_f[:])
            schb = consts.tile([CH, NH], f32, tag="schb")
            bcast_read(schb[:], scale_d[0, :], parts=CH)

            xs_r = xs.rearrange("b (s p) n -> b p s n", p=P)
            out_r = out_d.rearrange("b (s p) n -> b p s n", p=P)

            for img in range(BPC):
                st_dram = dramp.tile([2, N], f32, tag="st")
                st2_dram = dramp.tile([2, N], f32, tag="st2")
                nq_dram = dramp.tile([1, C], f32, tag="nq")

                xbf = resA.tile([P, KS, N], bf16, tag="xbf", bufs=1)
                invcol = resA.tile([P, NT], f32, tag="invc", bufs=2)
                ps_s = psacc.tile([CH, NH, CH], f32, tag="S")
                norms = psacc.tile([33, C], f32, tag="N")
                nq_ps = norms[0:1, :]
                nk_ps = norms[32:33, :]

                # ---------------- pass A: stats + qk + S/norm accum ------
                for f in range(NFG):
                    sl = slice(f * FG, (f + 1) * FG)
                    xc = work.tile([P, KS, FG], f32, tag="xcf")
                    nc.sync.dma_start(xc[:], xs_r[img][:, :, sl])
                    nc.vector.tensor_copy(xbf[:, :, sl], xc[:])
                    x8 = work.tile([P, 4, FG], f8, tag="x8")
                    if f < 2:
                        nc.gpsimd.memset(x8[:, 3, :], 0.0)
                    nc.vector.tensor_copy(x8[:, 0:KS, :], xc[:])
                    xsq = work.tile([P, 4, FG], f8, tag="xsq", bufs=2)
                    nc.vector.tensor_mul(xsq[:], x8[:], x8[:])
                    pst_a = ps.tile([1, FG], f32, tag="ps")
                    pst_b = ps.tile([1, FG], f32, tag="ps")
                    for s in range(KS):
                        nc.tensor.matmul(
                            pst_a[:], ones_f8[:], x8[:, s, :],
                            start=(s == 0), stop=(s == KS - 1))
                    for s in range(KS):
                        nc.tensor.matmul(
                            pst_b[:], ones_f8[:], xsq[:, s, :],
                            start=(s == 0), stop=(s == KS - 1))
                    srow = work.tile([1, 2, FG], f32, tag="srow")
                    nc.vector.tensor_copy(srow[0:1, 0, :], pst_a[:])
                    nc.vector.tensor_copy(srow[0:1, 1, :], pst_b[:])
                    nc.sync.dma_start(st_dram[:, sl], srow[:])
                    cstat = work.tile([P, 2, 4], f32, tag="cst")
                    for kk in range(2):
                        nc.gpsimd.dma_start(
                            cstat[:, kk, :],
                            st_dram[kk, sl].rearrange("(j p) -> p j", p=P))
                    mcol = work.tile([P, 4], f32, tag="mcol")
                    nc.vector.tensor_scalar(
                        mcol[:], cstat[:, 0, :], 1.0 / C, None, op0=ALU.mult)
                    vcol = work.tile([P, 4], f32, tag="vcol")
                    nc.vector.tensor_scalar(
                        vcol[:], cstat[:, 1, :], 1.0 / C, EPS_LN,
                        op0=ALU.mult, op1=ALU.add)
                    nc.vector.tensor_mul(mcol[:], mcol[:], mcol[:])
                    nc.vector.tensor_sub(vcol[:], vcol[:], mcol[:])
                    c4 = slice(4 * f, 4 * f + 4)
                    nc.vector.reciprocal(invcol[:, c4], vcol[:])

                    qsc8 = work.tile([P, 2, C], f8, tag="qsc8", bufs=2)
                    qkk8 = work.tile([P, 2, C], f8, tag="qkk8", bufs=2)
                    for t in range(4):
                        j = 4 * f + t
                        par = t % 2
                        pa = ps.tile([P, 512], f32, tag="ps")
                        pb = ps.tile([P, 256], f32, tag="ps")
                        lsl = slice(t * P, (t + 1) * P)
                        for pr in range(2):
                            pp2 = slice(2 * pr, 2 * pr + 2)
                            nc.tensor.matmul(
                                pa[:], x8[:, pp2, lsl], wqk_sb[:, pp2, 0:512],
                                start=(pr == 0), stop=(pr == 1), perf_mode=DR)
                            nc.tensor.matmul(
                                pb[:], x8[:, pp2, lsl], wqk_sb[:, pp2, 512:768],
                                start=(pr == 0), stop=(pr == 1), perf_mode=DR)
                        # evict plain q8 = q, k8 = k (frees PSUM fast), then
                        # qsc8 = q*inv, sqw = inv*q^2 | k^2 from SBUF only
                        q8 = work.tile([P, C], f8, tag="q8", bufs=2)
                        k8 = work.tile([P, C], f8, tag="k8", bufs=2)
                        nc.vector.tensor_scalar(
                            q8[:], pa[:, 0:C], 1.0 / 16.0, None, op0=ALU.mult)
                        nc.vector.tensor_scalar(
                            k8[:, 0:P], pa[:, C:512], 1.0 / 16.0,
                            None, op0=ALU.mult)
                        nc.vector.tensor_scalar(
                            k8[:, P:C], pb[:], 1.0 / 16.0, None, op0=ALU.mult)
                        nc.vector.tensor_scalar_mul(
                            qsc8[:, par, :], q8[:], invcol[:, j:j + 1])
                        nc.vector.tensor_copy(qkk8[:, par, :], k8[:])
                        sqw = work.tile([P, 2 * C], bf16, tag="sqw", bufs=2)
                        nc.vector.tensor_mul(
                            sqw[:, 0:C], qsc8[:, par, :], q8[:])
                        nc.vector.tensor_mul(sqw[:, C:2 * C], k8[:], k8[:])
                        st_, sp_ = (j == 0), (j == NT - 1)
                        nc.tensor.matmul(
                            nq_ps, ones_bf[:], sqw[:, 0:C],
                            start=st_, stop=sp_)
                        nc.tensor.matmul(
                            nk_ps, ones_bf[:], sqw[:, C:2 * C],
                            start=st_, stop=sp_)
                        if par == 1:
                            sS_, sP_ = (j == 1), (j == NT - 1)
                            for h in range(NH):
                                hs48 = slice(h * CH, (h + 1) * CH)
                                nc.tensor.matmul(
                                    ps_s[:, h, :],
                                    qsc8[:, :, hs48],
                                    qkk8[:, :, hs48],
                                    start=sS_, stop=sP_, perf_mode=DR)

                # ---------------- attention + G build --------------------
                nqrow = work.tile([1, C], f32, tag="nqrow", bufs=1)
                nc.vector.tensor_copy(nqrow[:], nq_ps)
                nc.sync.dma_start(nq_dram[:], nqrow[:])
                rqk = work.tile([CH, NH], f32, tag="rqk", bufs=1)
                nc.gpsimd.dma_start(
                    rqk[:], nq_dram.rearrange("a (h d) -> d (a h)", d=CH))
                rkrow = work.tile([1, C], f32, tag="rkrow", bufs=1)
                nc.scalar.activation(rkrow[:], nk_ps, AF.Sqrt)
                nc.vector.tensor_scalar_max(rkrow[:], rkrow[:], EPS_NORM)
                rki = work.tile([1, C], f32, tag="rki", bufs=1)
                nc.vector.reciprocal(rki[:], rkrow[:])
                rk_bf = work.tile([1, C], bf16, tag="rkbf", bufs=1)
                nc.vector.tensor_copy(rk_bf[:], rki[:])
                rkb = ps.tile([CH, C], f32, tag="ps")
                nc.tensor.matmul(
                    rkb[:], onesrow_bf[0:1, 0:CH], rk_bf[:],
                    start=True, stop=True)
                rqc = work.tile([CH, NH], f32, tag="rqc", bufs=1)
                nc.scalar.activation(rqc[:], rqk[:], AF.Sqrt)
                nc.vector.tensor_scalar_max(rqc[:], rqc[:], EPS_NORM)
                rqi = work.tile([CH, NH], f32, tag="rqi", bufs=1)
                nc.vector.reciprocal(rqi[:], rqc[:])
                nc.vector.tensor_mul(rqi[:], rqi[:], schb[:])
                sS = work.tile([CH, NH, CH], f32, tag="sS", bufs=1)
                nc.vector.tensor_mul(
                    sS[:], ps_s[:],
                    rqi[:, :, None].to_broadcast((CH, NH, CH)))
                rkb3 = rkb.rearrange("d (h e) -> d h e", e=CH)
                nc.vector.tensor_mul(sS[:], sS[:], rkb3)
                expS = work.tile([CH, NH, CH], f32, tag="expS", bufs=1)
                nc.scalar.activation(expS[:], sS[:], AF.Exp)
                esum = work.tile([CH, NH, 1], f32, tag="esum", bufs=1)
                nc.vector.reduce_sum(esum[:], expS[:], axis=AX.X)
                esi = work.tile([CH, NH, 1], f32, tag="esi", bufs=1)
                nc.vector.reciprocal(esi[:], esum[:])
                attn_bf = work.tile([CH, NH, CH], bf16, tag="attnb", bufs=1)
                nc.vector.tensor_mul(
                    attn_bf[:], expS[:], esi.to_broadcast((CH, NH, CH)))
                m1 = work.tile([CH, NH, C], bf16, tag="m1", bufs=1)
                for h in range(NH):
                    pm = ps.tile([CH, C], f32, tag="ps")
                    nc.tensor.matmul(
                        pm[:], attn_bf[:, h, :], wpj_sb[:, h, :],
                        start=True, stop=True)
                    nc.vector.tensor_copy(m1[:, h, :], pm[:])
                gbf = resA.tile([P, 4, C], f8, tag="gbf", bufs=2)
                nc.gpsimd.memset(gbf[:, 3, :], 0.0)
                for jc in range(KS):
                    pg = ps.tile([P, C], f32, tag="ps")
                    for h in range(NH):
                        nc.tensor.matmul(
                            pg[:], wv_sb[:, h, jc * P:(jc + 1) * P],
                            m1[:, h, :], start=(h == 0), stop=(h == NH - 1))
                    nc.vector.tensor_scalar(
                        gbf[:, jc, :], pg[:], 64.0, None, op0=ALU.mult)
                rstdc = work.tile([P, NT], bf16, tag="rstdc", bufs=1)
                nc.scalar.activation(rstdc[:], invcol[:], AF.Sqrt)
                psT = ps.tile([NT, P], bf16, tag="ps")
                nc.tensor.transpose(psT[:], rstdc[:], ident_bf[:])
                rstdT = work.tile([NT, P], bf16, tag="rstdT", bufs=1)
                nc.vector.tensor_copy(rstdT[:], psT[:])
                rstd_row = resA.tile([1, NT, P], bf16, tag="rstdrow", bufs=2)
                nc.gpsimd.dma_start(rstd_row[:], rstdT[:])

                # ---------------- pass B1: y = x + attn branch + stats ---
                ybf = resB.tile([P, KS, N], bf16, tag="ybf")
                for f in range(NFG):
                    sl = slice(f * FG, (f + 1) * FG)
                    psR = ps.tile([P, FG], f32, tag="ps")
                    nc.tensor.matmul(
                        psR[:], onesrow_bf[:],
                        rstd_row.rearrange("a j p -> a (j p)")[:, sl],
                        start=True, stop=True)
                    rb_sb = work.tile([P, FG], bf16, tag="rbsb", bufs=1)
                    nc.vector.tensor_scalar(
                        rb_sb[:], psR[:], 1.0 / 64.0, None, op0=ALU.mult)
                    x8b = work.tile([P, 4, FG], f8, tag="x8")
                    if f < 2:
                        nc.gpsimd.memset(x8b[:, 3, :], 0.0)
                    nc.vector.tensor_copy(x8b[:, 0:KS, :], xbf[:, :, sl])
                    for jc in range(KS):
                        px = ps.tile([P, FG], f32, tag="ps")
                        for pr in range(2):
                            pp2 = slice(2 * pr, 2 * pr + 2)
                            nc.tensor.matmul(
                                px[:], gbf[:, pp2, jc * P:(jc + 1) * P],
                                x8b[:, pp2, :],
                                start=(pr == 0), stop=(pr == 1), perf_mode=DR)
                        nc.vector.tensor_mul(ybf[:, jc, sl], px[:], rb_sb[:])
                        nc.vector.tensor_add(
                            ybf[:, jc, sl], ybf[:, jc, sl], xbf[:, jc, sl])
                    ysq = work.tile([P, KS, FG], bf16, tag="ysq")
                    nc.vector.tensor_mul(ysq[:], ybf[:, :, sl], ybf[:, :, sl])
                    pst_a = ps.tile([1, FG], f32, tag="ps")
                    pst_b = ps.tile([1, FG], f32, tag="ps")
                    for s in range(KS):
                        nc.tensor.matmul(
                            pst_a[:], ones_bf[:], ybf[:, s, sl],
                            start=(s == 0), stop=(s == KS - 1))
                    for s in range(KS):
                        nc.tensor.matmul(
                            pst_b[:], ones_bf[:], ysq[:, s, :],
                            start=(s == 0), stop=(s == KS - 1))
                    srow2 = work.tile([1, 2, FG], f32, tag="srow")
                    nc.vector.tensor_copy(srow2[0:1, 0, :], pst_a[:])
                    nc.vector.tensor_copy(srow2[0:1, 1, :], pst_b[:])
                    nc.sync.dma_start(st2_dram[:, sl], srow2[:])
                cst2 = work.tile([P, 2, NT], f32, tag="cst2", bufs=1)
                for kk in range(2):
                    nc.gpsimd.dma_start(
                        cst2[:, kk, :],
                        st2_dram[kk, :].rearrange("(j p) -> p j", p=P))
                mr2 = work.tile([P, 2, NT], f32, tag="mr2", bufs=1)
                nc.vector.tensor_scalar(
                    mr2[:, 0, :], cst2[:, 0, :], -1.0 / C, None, op0=ALU.mult)
                v2 = work.tile([P, NT], f32, tag="v2", bufs=1)
                nc.vector.tensor_scalar(
                    v2[:], cst2[:, 1, :], 1.0 / C, EPS_LN,
                    op0=ALU.mult, op1=ALU.add)
                msq2 = work.tile([P, NT], f32, tag="msq2", bufs=1)
                nc.vector.tensor_mul(msq2[:], mr2[:, 0, :], mr2[:, 0, :])
                nc.vector.tensor_sub(v2[:], v2[:], msq2[:])
                vi2 = work.tile([P, NT], f32, tag="vi2", bufs=1)
                nc.vector.reciprocal(vi2[:], v2[:])
                nc.scalar.activation(mr2[:, 1, :], vi2[:], AF.Sqrt, scale=256.0)
                nc.vector.tensor_mul(mr2[:, 0, :], mr2[:, 0, :], mr2[:, 1, :])
                mr2_bf = work.tile([P, 2, NT], bf16, tag="mr2b", bufs=1)
                nc.vector.tensor_copy(mr2_bf[:], mr2[:])
                psT2 = ps.tile([2 * NT, P], bf16, tag="ps")
                nc.tensor.transpose(
                    psT2[:], mr2_bf.rearrange("p two j -> p (two j)"),
                    ident_bf[:])
                m2T = work.tile([2 * NT, P], bf16, tag="m2T", bufs=1)
                nc.vector.tensor_copy(m2T[:], psT2[:])
                m2_row = resB.tile([1, 2, NT, P], bf16, tag="m2row", bufs=1)
                nc.gpsimd.dma_start(m2_row[:], m2T[:])

                # ---------------- pass B2: LN2 + FFN + residual ----------
                yn_tiles = {}

                def emit_ynprep(f):
                    sl = slice(f * FG, (f + 1) * FG)
                    bcM = ps.tile([P, FG], f32, tag="ps", name=f"bcM{f}")
                    bcR = ps.tile([P, FG], f32, tag="ps", name=f"bcR{f}")
                    m2f = m2_row.rearrange("a two j p -> a two (j p)")
                    nc.tensor.matmul(
                        bcM[:], onesrow_bf[:], m2f[:, 0, sl],
                        start=True, stop=True)
                    nc.tensor.matmul(
                        bcR[:], onesrow_bf[:], m2f[:, 1, sl],
                        start=True, stop=True)
                    t_yn = work.tile([P, KS, FG], bf16, tag="tyn", bufs=2,
                                     name=f"tyn{f}")
                    nc.vector.tensor_mul(
                        t_yn[:], ybf[:, :, sl],
                        bcR[:, None, :].to_broadcast((P, KS, FG)))
                    yn = work.tile([P, 4, FG], f8, tag="yn", name=f"yn{f}")
                    if img == 0 and f < 2:
                        nc.gpsimd.memset(yn[:, 3, :], 0.0)
                    nc.vector.tensor_add(
                        yn[:, 0:KS, :], t_yn[:],
                        bcM[:, None, :].to_broadcast((P, KS, FG)))
                    yn_tiles[f] = yn

                emit_ynprep(0)
                for f in range(NFG):
                    sl = slice(f * FG, (f + 1) * FG)
                    yn = yn_tiles.pop(f)
                    h_f8 = work.tile([P, KH, FG], f8, tag="h", bufs=1)
                    po_t = [ps.tile([P, FG], f32, tag="po", bufs=2,
                                    name=f"po{o}")
                            for o in range(2)]

                    def ffn2_pair(j2):
                        for o in range(2):
                            nc.tensor.matmul(
                                po_t[o][:],
                                w2_sb[:, 2 * j2:2 * j2 + 2, o * P:(o + 1) * P],
                                h_f8[:, 2 * j2:2 * j2 + 2, :],
                                start=(j2 == 0), stop=(j2 == KH // 2 - 1),
                                perf_mode=DR)

                    for m in range(KH):
                        ph = ps.tile([P, FG], f32, tag="ps")
                        for pr in range(2):
                            nc.tensor.matmul(
                                ph[:],
                                w1_sb[:, 2 * pr:2 * pr + 2, m * P:(m + 1) * P],
                                yn[:, 2 * pr:2 * pr + 2, :],
                                start=(pr == 0), stop=(pr == 1),
                                perf_mode=DR)
                        nc.scalar.activation(
                            h_f8[:, m, :], ph[:], AF.Gelu, scale=1.0 / 256.0)
                        if m == 6 and f + 1 < NFG:
                            emit_ynprep(f + 1)
                        if m >= 3 and (m - 3) % 2 == 0:
                            ffn2_pair((m - 3) // 2)
                    ffn2_pair(KH // 2 - 1)
                    po2 = ps.tile([P, FG], f32, tag="ps")
                    for j2 in range(KH // 2):
                        nc.tensor.matmul(
                            po2[:], w2_sb[:, 2 * j2:2 * j2 + 2, 2 * P:3 * P],
                            h_f8[:, 2 * j2:2 * j2 + 2, :],
                            start=(j2 == 0), stop=(j2 == KH // 2 - 1),
                            perf_mode=DR)
                    out_t = work.tile([P, KS, FG], f32, tag="xcf")
                    for o in range(2):
                        nc.vector.scalar_tensor_tensor(
                            out_t[:, o, :], po_t[o][:], 1.0 / 16.0,
                            ybf[:, o, sl], op0=ALU.mult, op1=ALU.add)
                    nc.vector.scalar_tensor_tensor(
                        out_t[:, 2, :], po2[:], 1.0 / 16.0,
                        ybf[:, 2, sl], op0=ALU.mult, op1=ALU.add)
                    nc.sync.dma_start(out_r[img][:, :, sl], out_t[:])
    return _split_waits(nc)


def _prep_weights(inputs):
    bf = ml_dtypes.bfloat16
    f8 = ml_dtypes.float8_e4m3fn
    w_qkv = np.asarray(inputs["w_qkv"], np.float64)
    g1 = np.asarray(inputs["g1"], np.float64)
    g2 = np.asarray(inputs["g2"], np.float64)
    for name in ("beta1", "beta2", "b_qkv", "b_proj", "b_ffn1", "b_ffn2"):
        assert not np.any(np.asarray(inputs[name])), f"{name} nonzero unsupported"
    wg = w_qkv * g1[None, :]
    wg = wg - wg.mean(axis=1, keepdims=True)  # fold LN mean-subtraction
    wg3 = wg.reshape(NH, 3 * CH, C)
    wq = wg3[:, 0:CH, :]
    wk = wg3[:, CH:2 * CH, :]
    wv_ = wg3[:, 2 * CH:3 * CH, :]
    # qk columns: all q heads first (384), then all k heads (384)
    wqk = np.concatenate(
        [wq.reshape(C, C), wk.reshape(C, C)], axis=0)  # [768, 384]
    wqk_r = np.zeros((P, 4, 2 * C), np.float64)  # K padded 384 -> 512
    wqk_r[:, 0:KS, :] = (16.0 * wqk).T.reshape(KS, P, 2 * C).transpose(1, 0, 2)
    wv_t = np.ascontiguousarray(wv_.transpose(1, 0, 2))  # [48, NH, 384]
    wpj = np.ascontiguousarray(
        np.asarray(inputs["w_proj"], np.float64).T.reshape(NH, CH, C)
        .transpose(1, 0, 2))  # [d, h, o]
    w1g = np.asarray(inputs["w_ffn1"], np.float64) * g2[None, :]
    w1g = w1g - w1g.mean(axis=1, keepdims=True)
    w1_r = np.zeros((P, 4, HID), np.float64)  # K padded 384 -> 512
    w1_r[:, 0:KS, :] = (16.0 * w1g).T.reshape(KS, P, HID).transpose(1, 0, 2)
    w2_r = np.ascontiguousarray(
        16.0 * np.asarray(inputs["w_ffn2"], np.float64).T
        .reshape(KH, P, C).transpose(1, 0, 2))  # [128, 12, 384]
    ls = np.asarray(inputs["logit_scale"], np.float32).reshape(NH)
    scale_row = np.exp(np.minimum(ls, LOGIT_MAX))[None, :]
    return dict(
        wqk=np.ascontiguousarray(wqk_r).astype(f8),
        wv=wv_t.astype(bf), wpj=wpj.astype(bf),
        w1=np.ascontiguousarray(w1_r).astype(f8), w2=w2_r.astype(f8),
        scale_row=np.ascontiguousarray(scale_row.astype(np.float32)))


def kernel(**inputs):
    from concourse.bass_utils import run_bass_kernel_spmd

    if "nc" not in _CACHE:
        _CACHE["nc"] = _build_nc()
    nc = _CACHE["nc"]

    x = np.asarray(inputs["x"], np.float32).reshape(B, C, N)
    wmap = _prep_weights(inputs)
    in_maps = []
    for c in range(NCORES):
        m = dict(wmap)
        m["xs"] = np.ascontiguousarray(x[c * BPC:(c + 1) * BPC])
        in_maps.append(m)
    res = run_bass_kernel_spmd(nc, in_maps, list(range(NCORES)))
    out = np.concatenate([r["out"] for r in res.results], axis=0)
    return out.reshape(B, C, 64, 64).astype(np.float32)


# revision 30
# speedup vs baseline: 1.3464x; 1.1034x over previous
"""Trainium2 Bass kernel for nn_CATransformer1 (XCiT-style channel-attention block).

v2: bf16 matmuls, LN centering folded into host-prepared weights, S-gram
weighted by inv-variance on the q side, transpose-free G build, fused
ffn1/ffn2 pipeline with F=512 moving tiles.

Sharding: data-parallel over batch. 16 images / 8 cores = 2 images per core.

Math (per image, x [C=384, N=4096]):
  LN1 gamma and the mean-subtraction are folded into the QKV weights on the
  host: W' = W*g1 - rowmean(W*g1) (exact because sum_c (x-m) = 0 per pixel).
  q,k are then produced directly from raw x; the per-pixel 1/std enters as
  a weight inv_n = 1/var_n on the pixel-contraction of the S-gram
  (S[c,d] = sum_n inv_n q_cn k_dn) and of the q/k norm sums.  Per-pixel
  stats are computed via ones-matmuls in row layout, round-tripped through
  DRAM into pixel-partition column layout for cheap vector postprocessing.
  The attention output + projection collapses into a per-image 384x384
  matrix G = Wproj @ concat_h(attn_h @ Wv_h) (Wv row-centered on the host, so
  G is automatically column-centered); pass B computes
  y = x + rstd ⊙ (G @ x) with rstd broadcast via ones-column matmuls.
  FFN: LN2 folded into W1'' = W1*g2 - rowmean likewise; yn = (y - m2)*rstd2
  materialized once per chunk in bf16; gelu on scalar engine; ffn2
  interleaved with ffn1 (lag 2) to keep the PE busy.
"""

import numpy as np
import ml_dtypes

B, C, NH, CH, N, HID = 16, 384, 8, 48, 4096, 1536
NCORES = 8
BPC = B // NCORES  # images per core
P = 128
KS = C // P    # 3 k-subtiles for C
KH = HID // P  # 12 k-subtiles for HID
FG = 512       # pixel chunk
NFG = N // FG  # 8
NT = N // P    # 32 128-pixel chunks
LOGIT_MAX = float(np.log(1.0 / 0.01))
EPS_LN = 1e-5
EPS_NORM = 1e-12

_CACHE = {}


def _patch_tile_drain():
    """Walrus in this env rejects >1 sync-wait on the kernel-tail Drain
    (CTRL_NO_STRUCT setupSyncWait).  Split the waits across a chain of
    drain instructions, one wait each.  Idempotent, in-process only."""
    import concourse.tile as tile
    from concourse import mybir
    from concourse.vector_clock import ScopedClock

    if getattr(tile.TileContext._drain_and_barrier, "_split_patch", False):
        return

    def _split_drain(self, tick_clock, wait_clock):
        drain_inst = self.nc.sync.drain()
        wait_clock.add_sem_waits(
            drain_inst.ins, ScopedClock({None: tick_clock.global_clock}))
        si = drain_inst.ins.sync_info
        if si is not None and si.on_wait and len(si.on_wait) > 1:
            waits = list(si.on_wait)
            si.on_wait = waits[:1]
            for w in waits[1:]:
                d2 = self.nc.sync.drain()
                d2.ins.sync_info = mybir.SyncInfo(on_wait=[w], on_update=[])
        self.nc.all_engine_barrier()
        popped = self.nc._tile_sem_poison_stack.pop()
        assert popped is self._sem_poison
        self.nc.clear_and_free_semaphores(list(self.sems.allocated().values()))
        self.nc.all_engine_barrier()

    _split_drain._split_patch = True
    tile.TileContext._drain_and_barrier = _split_drain


def _split_waits(nc, max_waits=1):
    """This walrus build rejects instructions carrying more than one sync
    wait ('Too many sync wait commands' / 'ISA wrong length').  Move extra
    waits onto same-engine NoOps inserted immediately before."""
    from concourse import mybir

    n = 0
    for fn in nc.m.functions:
        for blk in fn.blocks:
            out = []
            for inst in blk.instructions:
                si = inst.sync_info
                if si is not None and si.on_wait and len(si.on_wait) > max_waits:
                    waits = list(si.on_wait)
                    for w in waits[:-max_waits]:
                        n += 1
                        nop = mybir.InstNoOp(
                            name=f"I-wsplit-{n}", ins=[], outs=[])
                        nop.engine = inst.engine
                        nop.sync_info = mybir.SyncInfo(
                            on_wait=[w], on_update=[])
                        out.append(nop)
                    si.on_wait = waits[-max_waits:]
                out.append(inst)
            blk.instructions = out
    return nc


def _build_nc():
    import concourse.bass as bass
    import concourse.tile as tile
    from concourse import mybir
    from concourse.masks import make_identity

    dt = mybir.dt
    AF = mybir.ActivationFunctionType
    ALU = mybir.AluOpType
    AX = mybir.AxisListType

    f32 = dt.float32
    bf16 = dt.bfloat16
    f8 = dt.float8e4
    DR = mybir.MatmulPerfMode.DoubleRow

    _patch_tile_drain()
    nc = bass.Bass()

    xs = nc.declare_dram_parameter("xs", [BPC, C, N], f32, isOutput=False)
    wqk_d = nc.declare_dram_parameter("wqk", [P, 4, 2 * C], f8, isOutput=False)
    wv_d = nc.declare_dram_parameter("wv", [CH, NH, C], bf16, isOutput=False)
    wpj_d = nc.declare_dram_parameter("wpj", [CH, NH, C], bf16, isOutput=False)
    w1_d = nc.declare_dram_parameter("w1", [P, 4, HID], f8, isOutput=False)
    w2_d = nc.declare_dram_parameter("w2", [P, KH, C], f8, isOutput=False)
    scale_d = nc.declare_dram_parameter("scale_row", [1, NH], f32, isOutput=False)
    out_d = nc.declare_dram_parameter("out", [BPC, C, N], f32, isOutput=True)

    with tile.TileContext(nc) as tc:
        with (
            tc.tile_pool(name="consts", bufs=1) as consts,
            tc.tile_pool(name="resA", bufs=1) as resA,
            tc.tile_pool(name="resB", bufs=1) as resB,
            tc.tile_pool(name="work", bufs=2) as work,
            tc.tile_pool(name="ps", bufs=4, space="PSUM") as ps,
            tc.tile_pool(name="psacc", bufs=1, space="PSUM") as psacc,
            tc.tile_pool(name="dram", bufs=2, space="DRAM") as dramp,
        ):
            def bcast_read(dst, dram_row, parts):
                src = bass.AP(
                    tensor=dram_row.tensor, offset=dram_row.offset,
                    ap=[[0, parts]] + [list(d) for d in dram_row.ap[-1:]])
                nc.gpsimd.dma_start(dst, src)

            # ----------------- constants -----------------
            wqk_sb = consts.tile([P, 4, 2 * C], f8, tag="wqk")
            nc.scalar.dma_start(wqk_sb[:], wqk_d[:])
            wv_sb = consts.tile([CH, NH, C], bf16, tag="wv")
            nc.scalar.dma_start(wv_sb[:], wv_d[:])
            wpj_sb = consts.tile([CH, NH, C], bf16, tag="wpj")
            nc.scalar.dma_start(wpj_sb[:], wpj_d[:])
            w1_sb = consts.tile([P, 4, HID], f8, tag="w1")
            nc.scalar.dma_start(w1_sb[:], w1_d[:])
            w2_sb = consts.tile([P, KH, C], f8, tag="w2")
            nc.scalar.dma_start(w2_sb[:], w2_d[:])
            ones_f = consts.tile([P, 1], f32, tag="onesf")
            nc.vector.memset(ones_f[:], 1.0)
            ones_bf = consts.tile([P, 1], bf16, tag="ones")
            nc.vector.tensor_copy(ones_bf[:], ones_f[:])
            onesrow_f = consts.tile([1, P], f32, tag="onesrowf")
            nc.vector.memset(onesrow_f[:], 1.0)
            onesrow_bf = consts.tile([1, P], bf16, tag="onesrow")
            nc.vector.tensor_copy(onesrow_bf[:], onesrow_f[:])
            ident_bf = consts.tile([P, P], bf16, tag="ident")
            make_identity(nc, ident_bf[:])
            ones_f8 = consts.tile([P, 1], f8, tag="ones8")
            nc.vector.tensor_copy(ones_f8[:], ones_f[:])
            schb = consts.tile([CH, NH], f32, tag="schb")
            bcast_read(schb[:], scale_d[0, :], parts=CH)

            xs_r = xs.rearrange("b (s p) n -> b p s n", p=P)
            out_r = out_d.rearrange("b (s p) n -> b p s n", p=P)

            for img in range(BPC):
                st_dram = dramp.tile([2, N], f32, tag="st")
                st2_dram = dramp.tile([2, N], f32, tag="st2")
                nq_dram = dramp.tile([1, C], f32, tag="nq")

                invcol = resA.tile([P, NT], f32, tag="invc", bufs=2)
                ps_s = psacc.tile([CH, NH, CH], f32, tag="S")
                norms = psacc.tile([33, C], f32, tag="N")
                nq_ps = norms[0:1, :]
                nk_ps = norms[32:33, :]

                # ---------------- pass A: stats + qk + S/norm accum ------
                for f in range(NFG):
                    sl = slice(f * FG, (f + 1) * FG)
                    xc = work.tile([P, KS, FG], f32, tag="xcf")
                    nc.sync.dma_start(xc[:], xs_r[img][:, :, sl])
                    x8 = work.tile([P, 4, FG], f8, tag="x8")
                    if f < 2:
                        nc.gpsimd.memset(x8[:, 3, :], 0.0)
                    nc.vector.tensor_copy(x8[:, 0:KS, :], xc[:])
                    xsq = work.tile([P, 4, FG], f8, tag="xsq", bufs=2)
                    nc.vector.tensor_mul(xsq[:], x8[:], x8[:])
                    pst_a = ps.tile([1, FG], f32, tag="ps")
                    pst_b = ps.tile([1, FG], f32, tag="ps")
                    for s in range(KS):
                        nc.tensor.matmul(
                            pst_a[:], ones_f8[:], x8[:, s, :],
                            start=(s == 0), stop=(s == KS - 1))
                    for s in range(KS):
                        nc.tensor.matmul(
                            pst_b[:], ones_f8[:], xsq[:, s, :],
                            start=(s == 0), stop=(s == KS - 1))
                    srow = work.tile([1, 2, FG], f32, tag="srow")
                    nc.vector.tensor_copy(srow[0:1, 0, :], pst_a[:])
                    nc.vector.tensor_copy(srow[0:1, 1, :], pst_b[:])
                    nc.sync.dma_start(st_dram[:, sl], srow[:])
                    cstat = work.tile([P, 2, 4], f32, tag="cst")
                    for kk in range(2):
                        nc.gpsimd.dma_start(
                            cstat[:, kk, :],
                            st_dram[kk, sl].rearrange("(j p) -> p j", p=P))
                    mcol = work.tile([P, 4], f32, tag="mcol")
                    nc.vector.tensor_scalar(
                        mcol[:], cstat[:, 0, :], 1.0 / C, None, op0=ALU.mult)
                    vcol = work.tile([P, 4], f32, tag="vcol")
                    nc.vector.tensor_scalar(
                        vcol[:], cstat[:, 1, :], 1.0 / C, EPS_LN,
                        op0=ALU.mult, op1=ALU.add)
                    nc.vector.tensor_mul(mcol[:], mcol[:], mcol[:])
                    nc.vector.tensor_sub(vcol[:], vcol[:], mcol[:])
                    c4 = slice(4 * f, 4 * f + 4)
                    nc.vector.reciprocal(invcol[:, c4], vcol[:])

                    qsc8 = work.tile([P, 2, C], f8, tag="qsc8", bufs=2)
                    qkk8 = work.tile([P, 2, C], f8, tag="qkk8", bufs=2)
                    for t in range(4):
                        j = 4 * f + t
                        par = t % 2
                        pa = ps.tile([P, 512], f32, tag="ps")
                        pb = ps.tile([P, 256], f32, tag="ps")
                        lsl = slice(t * P, (t + 1) * P)
                        for pr in range(2):
                            pp2 = slice(2 * pr, 2 * pr + 2)
                            nc.tensor.matmul(
                                pa[:], x8[:, pp2, lsl], wqk_sb[:, pp2, 0:512],
                                start=(pr == 0), stop=(pr == 1), perf_mode=DR)
                            nc.tensor.matmul(
                                pb[:], x8[:, pp2, lsl], wqk_sb[:, pp2, 512:768],
                                start=(pr == 0), stop=(pr == 1), perf_mode=DR)
                        # evict plain q8 = q, k8 = k (frees PSUM fast), then
                        # qsc8 = q*inv, sqw = inv*q^2 | k^2 from SBUF only
                        q8 = work.tile([P, C], f8, tag="q8", bufs=2)
                        nc.vector.tensor_scalar(
                            q8[:], pa[:, 0:C], 1.0 / 16.0, None, op0=ALU.mult)
                        nc.vector.tensor_scalar(
                            qkk8[:, par, 0:P], pa[:, C:512], 1.0 / 16.0,
                            None, op0=ALU.mult)
                        nc.vector.tensor_scalar(
                            qkk8[:, par, P:C], pb[:], 1.0 / 16.0,
                            None, op0=ALU.mult)
                        nc.vector.tensor_scalar_mul(
                            qsc8[:, par, :], q8[:], invcol[:, j:j + 1])
                        sqw = work.tile([P, 2 * C], bf16, tag="sqw", bufs=2)
                        nc.vector.tensor_mul(
                            sqw[:, 0:C], qsc8[:, par, :], q8[:])
                        nc.vector.tensor_mul(
                            sqw[:, C:2 * C], qkk8[:, par, :], qkk8[:, par, :])
                        st_, sp_ = (j == 0), (j == NT - 1)
                        nc.tensor.matmul(
                            nq_ps, ones_bf[:], sqw[:, 0:C],
                            start=st_, stop=sp_)
                        nc.tensor.matmul(
                            nk_ps, ones_bf[:], sqw[:, C:2 * C],
                            start=st_, stop=sp_)
                        if par == 1:
                            sS_, sP_ = (j == 1), (j == NT - 1)
                            for h in range(NH):
                                hs48 = slice(h * CH, (h + 1) * CH)
                                nc.tensor.matmul(
                                    ps_s[:, h, :],
                                    qsc8[:, :, hs48],
                                    qkk8[:, :, hs48],
                                    start=sS_, stop=sP_, perf_mode=DR)

                # ---------------- attention + G build --------------------
                nqrow = work.tile([1, C], f32, tag="nqrow", bufs=1)
                nc.vector.tensor_copy(nqrow[:], nq_ps)
                nc.sync.dma_start(nq_dram[:], nqrow[:])
                rqk = work.tile([CH, NH], f32, tag="rqk", bufs=1)
                nc.gpsimd.dma_start(
                    rqk[:], nq_dram.rearrange("a (h d) -> d (a h)", d=CH))
                rkrow = work.tile([1, C], f32, tag="rkrow", bufs=1)
                nc.scalar.activation(rkrow[:], nk_ps, AF.Sqrt)
                nc.vector.tensor_scalar_max(rkrow[:], rkrow[:], EPS_NORM)
                rki = work.tile([1, C], f32, tag="rki", bufs=1)
                nc.vector.reciprocal(rki[:], rkrow[:])
                rk_bf = work.tile([1, C], bf16, tag="rkbf", bufs=1)
                nc.vector.tensor_copy(rk_bf[:], rki[:])
                rkb = ps.tile([CH, C], f32, tag="ps")
                nc.tensor.matmul(
                    rkb[:], onesrow_bf[0:1, 0:CH], rk_bf[:],
                    start=True, stop=True)
                rqc = work.tile([CH, NH], f32, tag="rqc", bufs=1)
                nc.scalar.activation(rqc[:], rqk[:], AF.Sqrt)
                nc.vector.tensor_scalar_max(rqc[:], rqc[:], EPS_NORM)
                rqi = work.tile([CH, NH], f32, tag="rqi", bufs=1)
                nc.vector.reciprocal(rqi[:], rqc[:])
                nc.vector.tensor_mul(rqi[:], rqi[:], schb[:])
                sS = work.tile([CH, NH, CH], f32, tag="sS", bufs=1)
                nc.vector.tensor_mul(
                    sS[:], ps_s[:],
                    rqi[:, :, None].to_broadcast((CH, NH, CH)))
                rkb3 = rkb.rearrange("d (h e) -> d h e", e=CH)
                nc.vector.tensor_mul(sS[:], sS[:], rkb3)
                expS = work.tile([CH, NH, CH], f32, tag="expS", bufs=1)
                nc.scalar.activation(expS[:], sS[:], AF.Exp)
                esum = work.tile([CH, NH, 1], f32, tag="esum", bufs=1)
                nc.vector.reduce_sum(esum[:], expS[:], axis=AX.X)
                esi = work.tile([CH, NH, 1], f32, tag="esi", bufs=1)
                nc.vector.reciprocal(esi[:], esum[:])
                attn_bf = work.tile([CH, NH, CH], bf16, tag="attnb", bufs=1)
                nc.vector.tensor_mul(
                    attn_bf[:], expS[:], esi.to_broadcast((CH, NH, CH)))
                m1 = work.tile([CH, NH, C], bf16, tag="m1", bufs=1)
                for h in range(NH):
                    pm = ps.tile([CH, C], f32, tag="ps")
                    nc.tensor.matmul(
                        pm[:], attn_bf[:, h, :], wpj_sb[:, h, :],
                        start=True, stop=True)
                    nc.vector.tensor_copy(m1[:, h, :], pm[:])
                gbf = resA.tile([P, 4, C], f8, tag="gbf", bufs=2)
                nc.gpsimd.memset(gbf[:, 3, :], 0.0)
                for jc in range(KS):
                    pg = ps.tile([P, C], f32, tag="ps")
                    for h in range(NH):
                        nc.tensor.matmul(
                            pg[:], wv_sb[:, h, jc * P:(jc + 1) * P],
                            m1[:, h, :], start=(h == 0), stop=(h == NH - 1))
                    nc.vector.tensor_scalar(
                        gbf[:, jc, :], pg[:], 64.0, None, op0=ALU.mult)
                rstdc = work.tile([P, NT], bf16, tag="rstdc", bufs=1)
                nc.scalar.activation(rstdc[:], invcol[:], AF.Sqrt)
                psT = ps.tile([NT, P], bf16, tag="ps")
                nc.tensor.transpose(psT[:], rstdc[:], ident_bf[:])
                rstdT = work.tile([NT, P], bf16, tag="rstdT", bufs=1)
                nc.vector.tensor_copy(rstdT[:], psT[:])
                rstd_row = resA.tile([1, NT, P], bf16, tag="rstdrow", bufs=2)
                nc.gpsimd.dma_start(rstd_row[:], rstdT[:])

                # ---------------- pass B1: y = x + attn branch + stats ---
                ybf = resB.tile([P, KS, N], bf16, tag="ybf")

                def emit_b1_stats(f):
                    sl = slice(f * FG, (f + 1) * FG)
                    ysq = work.tile([P, KS, FG], bf16, tag="ysq",
                                    name=f"ysq{f}")
                    nc.vector.tensor_mul(ysq[:], ybf[:, :, sl], ybf[:, :, sl])
                    pst_a = ps.tile([1, FG], f32, tag="ps", name=f"bsta{f}")
                    pst_b = ps.tile([1, FG], f32, tag="ps", name=f"bstb{f}")
                    for s in range(KS):
                        nc.tensor.matmul(
                            pst_a[:], ones_bf[:], ybf[:, s, sl],
                            start=(s == 0), stop=(s == KS - 1))
                    for s in range(KS):
                        nc.tensor.matmul(
                            pst_b[:], ones_bf[:], ysq[:, s, :],
                            start=(s == 0), stop=(s == KS - 1))
                    srow2 = work.tile([1, 2, FG], f32, tag="srow",
                                      name=f"srow2{f}")
                    nc.vector.tensor_copy(srow2[0:1, 0, :], pst_a[:])
                    nc.vector.tensor_copy(srow2[0:1, 1, :], pst_b[:])
                    nc.sync.dma_start(st2_dram[:, sl], srow2[:])

                for f in range(NFG):
                    sl = slice(f * FG, (f + 1) * FG)
                    xc2 = work.tile([P, KS, FG], f32, tag="xcf", name=f"xc2{f}")
                    nc.sync.dma_start(xc2[:], xs_r[img][:, :, sl])
                    psR = ps.tile([P, FG], f32, tag="ps")
                    nc.tensor.matmul(
                        psR[:], onesrow_bf[:],
                        rstd_row.rearrange("a j p -> a (j p)")[:, sl],
                        start=True, stop=True)
                    rb_sb = work.tile([P, FG], bf16, tag="rbsb", bufs=1)
                    nc.vector.tensor_scalar(
                        rb_sb[:], psR[:], 1.0 / 64.0, None, op0=ALU.mult)
                    x8b = work.tile([P, 4, FG], f8, tag="x8")
                    if f < 2:
                        nc.gpsimd.memset(x8b[:, 3, :], 0.0)
                    nc.vector.tensor_copy(x8b[:, 0:KS, :], xc2[:])
                    for jc in range(KS):
                        px = ps.tile([P, FG], f32, tag="ps")
                        for pr in range(2):
                            pp2 = slice(2 * pr, 2 * pr + 2)
                            nc.tensor.matmul(
                                px[:], gbf[:, pp2, jc * P:(jc + 1) * P],
                                x8b[:, pp2, :],
                                start=(pr == 0), stop=(pr == 1), perf_mode=DR)
                        nc.vector.tensor_mul(ybf[:, jc, sl], px[:], rb_sb[:])
                        nc.gpsimd.tensor_add(
                            ybf[:, jc, sl], ybf[:, jc, sl], xc2[:, jc, :])
                    if f > 0:
                        emit_b1_stats(f - 1)
                emit_b1_stats(NFG - 1)
                cst2 = work.tile([P, 2, NT], f32, tag="cst2", bufs=1)
                for kk in range(2):
                    nc.gpsimd.dma_start(
                        cst2[:, kk, :],
                        st2_dram[kk, :].rearrange("(j p) -> p j", p=P))
                mr2 = work.tile([P, 2, NT], f32, tag="mr2", bufs=1)
                nc.vector.tensor_scalar(
                    mr2[:, 0, :], cst2[:, 0, :], -1.0 / C, None, op0=ALU.mult)
                v2 = work.tile([P, NT], f32, tag="v2", bufs=1)
                nc.vector.tensor_scalar(
                    v2[:], cst2[:, 1, :], 1.0 / C, EPS_LN,
                    op0=ALU.mult, op1=ALU.add)
                msq2 = work.tile([P, NT], f32, tag="msq2", bufs=1)
                nc.vector.tensor_mul(msq2[:], mr2[:, 0, :], mr2[:, 0, :])
                nc.vector.tensor_sub(v2[:], v2[:], msq2[:])
                vi2 = work.tile([P, NT], f32, tag="vi2", bufs=1)
                nc.vector.reciprocal(vi2[:], v2[:])
                nc.scalar.activation(mr2[:, 1, :], vi2[:], AF.Sqrt, scale=256.0)
                nc.vector.tensor_mul(mr2[:, 0, :], mr2[:, 0, :], mr2[:, 1, :])
                mr2_bf = work.tile([P, 2, NT], bf16, tag="mr2b", bufs=1)
                nc.vector.tensor_copy(mr2_bf[:], mr2[:])
                psT2 = ps.tile([2 * NT, P], bf16, tag="ps")
                nc.tensor.transpose(
                    psT2[:], mr2_bf.rearrange("p two j -> p (two j)"),
                    ident_bf[:])
                m2T = work.tile([2 * NT, P], bf16, tag="m2T", bufs=1)
                nc.vector.tensor_copy(m2T[:], psT2[:])
                m2_row = resB.tile([1, 2, NT, P], bf16, tag="m2row", bufs=1)
                nc.gpsimd.dma_start(m2_row[:], m2T[:])

                # ---------------- pass B2: LN2 + FFN + residual ----------
                yn_tiles = {}

                def emit_ynprep(f):
                    sl = slice(f * FG, (f + 1) * FG)
                    bcM = ps.tile([P, FG], f32, tag="ps", name=f"bcM{f}")
                    bcR = ps.tile([P, FG], f32, tag="ps", name=f"bcR{f}")
                    m2f = m2_row.rearrange("a two j p -> a two (j p)")
                    nc.tensor.matmul(
                        bcM[:], onesrow_bf[:], m2f[:, 0, sl],
                        start=True, stop=True)
                    nc.tensor.matmul(
                        bcR[:], onesrow_bf[:], m2f[:, 1, sl],
                        start=True, stop=True)
                    t_yn = work.tile([P, KS, FG], bf16, tag="tyn", bufs=2,
                                     name=f"tyn{f}")
                    nc.vector.tensor_mul(
                        t_yn[:], ybf[:, :, sl],
                        bcR[:, None, :].to_broadcast((P, KS, FG)))
                    yn = work.tile([P, 4, FG], f8, tag="yn", name=f"yn{f}")
                    if img == 0 and f < 2:
                        nc.gpsimd.memset(yn[:, 3, :], 0.0)
                    nc.vector.tensor_add(
                        yn[:, 0:KS, :], t_yn[:],
                        bcM[:, None, :].to_broadcast((P, KS, FG)))
                    yn_tiles[f] = yn

                emit_ynprep(0)
                for f in range(NFG):
                    sl = slice(f * FG, (f + 1) * FG)
                    yn = yn_tiles.pop(f)
                    h_f8 = work.tile([P, KH, FG], f8, tag="h", bufs=1)
                    po_t = [ps.tile([P, FG], f32, tag="po", bufs=2,
                                    name=f"po{o}")
                            for o in range(2)]

                    def ffn2_pair(j2):
                        for o in range(2):
                            nc.tensor.matmul(
                                po_t[o][:],
                                w2_sb[:, 2 * j2:2 * j2 + 2, o * P:(o + 1) * P],
                                h_f8[:, 2 * j2:2 * j2 + 2, :],
                                start=(j2 == 0), stop=(j2 == KH // 2 - 1),
                                perf_mode=DR)

                    for m in range(KH):
                        ph = ps.tile([P, FG], f32, tag="ps")
                        for pr in range(2):
                            nc.tensor.matmul(
                                ph[:],
                                w1_sb[:, 2 * pr:2 * pr + 2, m * P:(m + 1) * P],
                                yn[:, 2 * pr:2 * pr + 2, :],
                                start=(pr == 0), stop=(pr == 1),
                                perf_mode=DR)
                        nc.scalar.activation(
                            h_f8[:, m, :], ph[:], AF.Gelu, scale=1.0 / 256.0)
                        if m == 6 and f + 1 < NFG:
                            emit_ynprep(f + 1)
                        if m >= 3 and (m - 3) % 2 == 0:
                            ffn2_pair((m - 3) // 2)
                    ffn2_pair(KH // 2 - 1)
                    po2 = ps.tile([P, FG], f32, tag="ps")
                    for j2 in range(KH // 2):
                        nc.tensor.matmul(
                            po2[:], w2_sb[:, 2 * j2:2 * j2 + 2, 2 * P:3 * P],
                            h_f8[:, 2 * j2:2 * j2 + 2, :],
                            start=(j2 == 0), stop=(j2 == KH // 2 - 1),
                            perf_mode=DR)
                    out_t = work.tile([P, KS, FG], f32, tag="xcf")
                    for o in range(2):
                        nc.vector.scalar_tensor_tensor(
                            out_t[:, o, :], po_t[o][:], 1.0 / 16.0,
                            ybf[:, o, sl], op0=ALU.mult, op1=ALU.add)
                    nc.vector.scalar_tensor_tensor(
                        out_t[:, 2, :], po2[:], 1.0 / 16.0,
                        ybf[:, 2, sl], op0=ALU.mult, op1=ALU.add)
                    nc.sync.dma_start(out_r[img][:, :, sl], out_t[:])
    return _split_waits(nc)


def _prep_weights(inputs):
    bf = ml_dtypes.bfloat16
    f8 = ml_dtypes.float8_e4m3fn
    w_qkv = np.asarray(inputs["w_qkv"], np.float64)
    g1 = np.asarray(inputs["g1"], np.float64)
    g2 = np.asarray(inputs["g2"], np.float64)
    for name in ("beta1", "beta2", "b_qkv", "b_proj", "b_ffn1", "b_ffn2"):
        assert not np.any(np.asarray(inputs[name])), f"{name} nonzero unsupported"
    wg = w_qkv * g1[None, :]
    wg = wg - wg.mean(axis=1, keepdims=True)  # fold LN mean-subtraction
    wg3 = wg.reshape(NH, 3 * CH, C)
    wq = wg3[:, 0:CH, :]
    wk = wg3[:, CH:2 * CH, :]
    wv_ = wg3[:, 2 * CH:3 * CH, :]
    # qk columns: all q heads first (384), then all k heads (384)
    wqk = np.concatenate(
        [wq.reshape(C, C), wk.reshape(C, C)], axis=0)  # [768, 384]
    wqk_r = np.zeros((P, 4, 2 * C), np.float64)  # K padded 384 -> 512
    wqk_r[:, 0:KS, :] = (16.0 * wqk).T.reshape(KS, P, 2 * C).transpose(1, 0, 2)
    wv_t = np.ascontiguousarray(wv_.transpose(1, 0, 2))  # [48, NH, 384]
    wpj = np.ascontiguousarray(
        np.asarray(inputs["w_proj"], np.float64).T.reshape(NH, CH, C)
        .transpose(1, 0, 2))  # [d, h, o]
    w1g = np.asarray(inputs["w_ffn1"], np.float64) * g2[None, :]
    w1g = w1g - w1g.mean(axis=1, keepdims=True)
    w1_r = np.zeros((P, 4, HID), np.float64)  # K padded 384 -> 512
    w1_r[:, 0:KS, :] = (16.0 * w1g).T.reshape(KS, P, HID).transpose(1, 0, 2)
    w2_r = np.ascontiguousarray(
        16.0 * np.asarray(inputs["w_ffn2"], np.float64).T
        .reshape(KH, P, C).transpose(1, 0, 2))  # [128, 12, 384]
    ls = np.asarray(inputs["logit_scale"], np.float32).reshape(NH)
    scale_row = np.exp(np.minimum(ls, LOGIT_MAX))[None, :]
    return dict(
        wqk=np.ascontiguousarray(wqk_r).astype(f8),
        wv=wv_t.astype(bf), wpj=wpj.astype(bf),
        w1=np.ascontiguousarray(w1_r).astype(f8), w2=w2_r.astype(f8),
        scale_row=np.ascontiguousarray(scale_row.astype(np.float32)))


def kernel(**inputs):
    from concourse.bass_utils import run_bass_kernel_spmd

    if "nc" not in _CACHE:
        _CACHE["nc"] = _build_nc()
    nc = _CACHE["nc"]

    x = np.asarray(inputs["x"], np.float32).reshape(B, C, N)
    wmap = _prep_weights(inputs)
    in_maps = []
    for c in range(NCORES):
        m = dict(wmap)
        m["xs"] = np.ascontiguousarray(x[c * BPC:(c + 1) * BPC])
        in_maps.append(m)
    res = run_bass_kernel_spmd(nc, in_maps, list(range(NCORES)))
    out = np.concatenate([r["out"] for r in res.results], axis=0)
    return out.reshape(B, C, 64, 64).astype(np.float32)


# revision 32
# speedup vs baseline: 1.3632x; 1.0125x over previous
"""Trainium2 Bass kernel for nn_CATransformer1 (XCiT-style channel-attention block).

v2: bf16 matmuls, LN centering folded into host-prepared weights, S-gram
weighted by inv-variance on the q side, transpose-free G build, fused
ffn1/ffn2 pipeline with F=512 moving tiles.

Sharding: data-parallel over batch. 16 images / 8 cores = 2 images per core.

Math (per image, x [C=384, N=4096]):
  LN1 gamma and the mean-subtraction are folded into the QKV weights on the
  host: W' = W*g1 - rowmean(W*g1) (exact because sum_c (x-m) = 0 per pixel).
  q,k are then produced directly from raw x; the per-pixel 1/std enters as
  a weight inv_n = 1/var_n on the pixel-contraction of the S-gram
  (S[c,d] = sum_n inv_n q_cn k_dn) and of the q/k norm sums.  Per-pixel
  stats are computed via ones-matmuls in row layout, round-tripped through
  DRAM into pixel-partition column layout for cheap vector postprocessing.
  The attention output + projection collapses into a per-image 384x384
  matrix G = Wproj @ concat_h(attn_h @ Wv_h) (Wv row-centered on the host, so
  G is automatically column-centered); pass B computes
  y = x + rstd ⊙ (G @ x) with rstd broadcast via ones-column matmuls.
  FFN: LN2 folded into W1'' = W1*g2 - rowmean likewise; yn = (y - m2)*rstd2
  materialized once per chunk in bf16; gelu on scalar engine; ffn2
  interleaved with ffn1 (lag 2) to keep the PE busy.
"""

import numpy as np
import ml_dtypes

B, C, NH, CH, N, HID = 16, 384, 8, 48, 4096, 1536
NCORES = 8
BPC = B // NCORES  # images per core
P = 128
KS = C // P    # 3 k-subtiles for C
KH = HID // P  # 12 k-subtiles for HID
FG = 512       # pixel chunk
NFG = N // FG  # 8
NT = N // P    # 32 128-pixel chunks
LOGIT_MAX = float(np.log(1.0 / 0.01))
EPS_LN = 1e-5
EPS_NORM = 1e-12

_CACHE = {}


def _patch_tile_drain():
    """Walrus in this env rejects >1 sync-wait on the kernel-tail Drain
    (CTRL_NO_STRUCT setupSyncWait).  Split the waits across a chain of
    drain instructions, one wait each.  Idempotent, in-process only."""
    import concourse.tile as tile
    from concourse import mybir
    from concourse.vector_clock import ScopedClock

    if getattr(tile.TileContext._drain_and_barrier, "_split_patch", False):
        return

    def _split_drain(self, tick_clock, wait_clock):
        drain_inst = self.nc.sync.drain()
        wait_clock.add_sem_waits(
            drain_inst.ins, ScopedClock({None: tick_clock.global_clock}))
        si = drain_inst.ins.sync_info
        if si is not None and si.on_wait and len(si.on_wait) > 1:
            waits = list(si.on_wait)
            si.on_wait = waits[:1]
            for w in waits[1:]:
                d2 = self.nc.sync.drain()
                d2.ins.sync_info = mybir.SyncInfo(on_wait=[w], on_update=[])
        self.nc.all_engine_barrier()
        popped = self.nc._tile_sem_poison_stack.pop()
        assert popped is self._sem_poison
        self.nc.clear_and_free_semaphores(list(self.sems.allocated().values()))
        self.nc.all_engine_barrier()

    _split_drain._split_patch = True
    tile.TileContext._drain_and_barrier = _split_drain


def _split_waits(nc, max_waits=1):
    """This walrus build rejects instructions carrying more than one sync
    wait ('Too many sync wait commands' / 'ISA wrong length').  Move extra
    waits onto same-engine NoOps inserted immediately before."""
    from concourse import mybir

    n = 0
    for fn in nc.m.functions:
        for blk in fn.blocks:
            out = []
            for inst in blk.instructions:
                si = inst.sync_info
                if si is not None and si.on_wait and len(si.on_wait) > max_waits:
                    waits = list(si.on_wait)
                    for w in waits[:-max_waits]:
                        n += 1
                        nop = mybir.InstNoOp(
                            name=f"I-wsplit-{n}", ins=[], outs=[])
                        nop.engine = inst.engine
                        nop.sync_info = mybir.SyncInfo(
                            on_wait=[w], on_update=[])
                        out.append(nop)
                    si.on_wait = waits[-max_waits:]
                out.append(inst)
            blk.instructions = out
    return nc


def _build_nc():
    import concourse.bass as bass
    import concourse.tile as tile
    from concourse import mybir
    from concourse.masks import make_identity

    dt = mybir.dt
    AF = mybir.ActivationFunctionType
    ALU = mybir.AluOpType
    AX = mybir.AxisListType

    f32 = dt.float32
    bf16 = dt.bfloat16
    f8 = dt.float8e4
    DR = mybir.MatmulPerfMode.DoubleRow

    _patch_tile_drain()
    nc = bass.Bass()

    xs = nc.declare_dram_parameter("xs", [BPC, C, N], f32, isOutput=False)
    wqk_d = nc.declare_dram_parameter("wqk", [P, 4, 2 * C], f8, isOutput=False)
    wv_d = nc.declare_dram_parameter("wv", [CH, NH, C], bf16, isOutput=False)
    wpj_d = nc.declare_dram_parameter("wpj", [CH, NH, C], bf16, isOutput=False)
    w1_d = nc.declare_dram_parameter("w1", [P, 4, HID], f8, isOutput=False)
    w2_d = nc.declare_dram_parameter("w2", [P, KH, C], f8, isOutput=False)
    scale_d = nc.declare_dram_parameter("scale_row", [1, NH], f32, isOutput=False)
    out_d = nc.declare_dram_parameter("out", [BPC, C, N], f32, isOutput=True)

    with tile.TileContext(nc) as tc:
        with (
            tc.tile_pool(name="consts", bufs=1) as consts,
            tc.tile_pool(name="resA", bufs=1) as resA,
            tc.tile_pool(name="resB", bufs=1) as resB,
            tc.tile_pool(name="work", bufs=2) as work,
            tc.tile_pool(name="ps", bufs=4, space="PSUM") as ps,
            tc.tile_pool(name="psacc", bufs=1, space="PSUM") as psacc,
            tc.tile_pool(name="dram", bufs=2, space="DRAM") as dramp,
        ):
            def bcast_read(dst, dram_row, parts):
                src = bass.AP(
                    tensor=dram_row.tensor, offset=dram_row.offset,
                    ap=[[0, parts]] + [list(d) for d in dram_row.ap[-1:]])
                nc.gpsimd.dma_start(dst, src)

            # ----------------- constants -----------------
            wqk_sb = consts.tile([P, 4, 2 * C], f8, tag="wqk")
            nc.scalar.dma_start(wqk_sb[:], wqk_d[:])
            wv_sb = consts.tile([CH, NH, C], bf16, tag="wv")
            nc.scalar.dma_start(wv_sb[:], wv_d[:])
            wpj_sb = consts.tile([CH, NH, C], bf16, tag="wpj")
            nc.scalar.dma_start(wpj_sb[:], wpj_d[:])
            w1_sb = consts.tile([P, 4, HID], f8, tag="w1")
            nc.scalar.dma_start(w1_sb[:], w1_d[:])
            w2_sb = consts.tile([P, KH, C], f8, tag="w2")
            nc.scalar.dma_start(w2_sb[:], w2_d[:])
            ones_f = consts.tile([P, 1], f32, tag="onesf")
            nc.vector.memset(ones_f[:], 1.0)
            ones_bf = consts.tile([P, 1], bf16, tag="ones")
            nc.vector.tensor_copy(ones_bf[:], ones_f[:])
            onesrow_f = consts.tile([1, P], f32, tag="onesrowf")
            nc.vector.memset(onesrow_f[:], 1.0)
            onesrow_bf = consts.tile([1, P], bf16, tag="onesrow")
            nc.vector.tensor_copy(onesrow_bf[:], onesrow_f[:])
            ident_bf = consts.tile([P, P], bf16, tag="ident")
            make_identity(nc, ident_bf[:])
            ones_f8 = consts.tile([P, 1], f8, tag="ones8")
            nc.vector.tensor_copy(ones_f8[:], ones_f[:])
            schb = consts.tile([CH, NH], f32, tag="schb")
            bcast_read(schb[:], scale_d[0, :], parts=CH)

            xs_r = xs.rearrange("b (s p) n -> b p s n", p=P)
            out_r = out_d.rearrange("b (s p) n -> b p s n", p=P)

            for img in range(BPC):
                st_dram = dramp.tile([2, N], f32, tag="st")
                st2_dram = dramp.tile([2, N], f32, tag="st2")
                nq_dram = dramp.tile([1, C], f32, tag="nq")

                invcol = resA.tile([P, NT], f32, tag="invc", bufs=2)
                inv_bf = resA.tile([P, NT], bf16, tag="invb", bufs=2)
                ps_s = psacc.tile([CH, NH, CH], f32, tag="S")
                norms = psacc.tile([33, C], f32, tag="N")
                nq_ps = norms[0:1, :]
                nk_ps = norms[32:33, :]

                # ---------------- pass A: stats + qk + S/norm accum ------
                xc_t, x8_t = {}, {}

                def emit_xc(f):
                    sl = slice(f * FG, (f + 1) * FG)
                    xc = work.tile([P, KS, FG], f32, tag="xcf", bufs=3,
                                   name=f"xc{f}")
                    nc.sync.dma_start(xc[:], xs_r[img][:, :, sl])
                    xc_t[f] = xc

                def emit_cast(f):
                    xc = xc_t.pop(f)
                    x8 = work.tile([P, 4, FG], f8, tag="x8", name=f"x8_{f}")
                    if img == 0 and f < 2:
                        nc.gpsimd.memset(x8[:, 3, :], 0.0)
                    nc.vector.tensor_copy(x8[:, 0:KS, :], xc[:])
                    xsq = work.tile([P, 4, FG], f8, tag="xsq", bufs=2,
                                    name=f"xsq{f}")
                    nc.vector.tensor_mul(xsq[:], x8[:], x8[:])
                    x8_t[f] = (x8, xsq)

                emit_xc(0)
                emit_xc(1)
                emit_cast(0)
                for f in range(NFG):
                    sl = slice(f * FG, (f + 1) * FG)
                    if f + 2 < NFG:
                        emit_xc(f + 2)
                    if f + 1 < NFG:
                        emit_cast(f + 1)
                    x8, xsq = x8_t.pop(f)
                    pst_a = ps.tile([1, FG], f32, tag="ps")
                    pst_b = ps.tile([1, FG], f32, tag="ps")
                    for s in range(KS):
                        nc.tensor.matmul(
                            pst_a[:], ones_f8[:], x8[:, s, :],
                            start=(s == 0), stop=(s == KS - 1))
                    for s in range(KS):
                        nc.tensor.matmul(
                            pst_b[:], ones_f8[:], xsq[:, s, :],
                            start=(s == 0), stop=(s == KS - 1))
                    srow = work.tile([1, 2, FG], f32, tag="srow")
                    nc.scalar.copy(srow[0:1, 0, :], pst_a[:])
                    nc.scalar.copy(srow[0:1, 1, :], pst_b[:])
                    nc.sync.dma_start(st_dram[:, sl], srow[:])
                    cstat = work.tile([P, 2, 4], f32, tag="cst")
                    for kk in range(2):
                        nc.gpsimd.dma_start(
                            cstat[:, kk, :],
                            st_dram[kk, sl].rearrange("(j p) -> p j", p=P))
                    mcol = work.tile([P, 4], f32, tag="mcol")
                    nc.vector.tensor_scalar(
                        mcol[:], cstat[:, 0, :], 1.0 / C, None, op0=ALU.mult)
                    vcol = work.tile([P, 4], f32, tag="vcol")
                    nc.vector.tensor_scalar(
                        vcol[:], cstat[:, 1, :], 1.0 / C, EPS_LN,
                        op0=ALU.mult, op1=ALU.add)
                    nc.vector.tensor_mul(mcol[:], mcol[:], mcol[:])
                    nc.vector.tensor_sub(vcol[:], vcol[:], mcol[:])
                    c4 = slice(4 * f, 4 * f + 4)
                    nc.vector.reciprocal(invcol[:, c4], vcol[:])
                    nc.vector.tensor_copy(inv_bf[:, c4], invcol[:, c4])

                    qk8p = [None, None]
                    qsc8p = [None, None]
                    sqt = [None] * 4
                    for t in range(4):
                        j = 4 * f + t
                        pr2 = t // 2
                        par = t % 2
                        if par == 0:
                            qk8p[pr2] = work.tile(
                                [P, 2, 2 * C], f8, tag="qk8p", bufs=2,
                                name=f"qk8p{f}_{pr2}")
                            qsc8p[pr2] = work.tile(
                                [P, 2, C], f8, tag="qsc8", bufs=2,
                                name=f"qsc8p{f}_{pr2}")
                        qk8raw, qsc8 = qk8p[pr2], qsc8p[pr2]
                        pa = ps.tile([P, 512], f32, tag="ps")
                        pb = ps.tile([P, 256], f32, tag="ps")
                        lsl = slice(t * P, (t + 1) * P)
                        for pr in range(2):
                            pp2 = slice(2 * pr, 2 * pr + 2)
                            nc.tensor.matmul(
                                pa[:], x8[:, pp2, lsl], wqk_sb[:, pp2, 0:512],
                                start=(pr == 0), stop=(pr == 1), perf_mode=DR)
                            nc.tensor.matmul(
                                pb[:], x8[:, pp2, lsl], wqk_sb[:, pp2, 512:768],
                                start=(pr == 0), stop=(pr == 1), perf_mode=DR)
                        nc.vector.tensor_scalar(
                            qk8raw[:, par, 0:512], pa[:], 1.0 / 16.0,
                            None, op0=ALU.mult)
                        nc.vector.tensor_scalar(
                            qk8raw[:, par, 512:768], pb[:], 1.0 / 16.0,
                            None, op0=ALU.mult)
                        nc.vector.tensor_scalar(
                            qsc8[:, par, :], pa[:, 0:C], invcol[:, j:j + 1],
                            1.0 / 16.0, op0=ALU.mult, op1=ALU.mult)
                        sq_bf = work.tile([P, 2 * C], bf16, tag="sqbf",
                                          bufs=3, name=f"sq{f}_{t}")
                        nc.vector.tensor_mul(
                            sq_bf[:], qk8raw[:, par, :], qk8raw[:, par, :])
                        sqt[t] = sq_bf
                    for t in range(4):
                        j = 4 * f + t
                        st_, sp_ = (j == 0), (j == NT - 1)
                        nc.tensor.matmul(
                            nq_ps, inv_bf[:, j:j + 1], sqt[t][:, 0:C],
                            start=st_, stop=sp_)
                        nc.tensor.matmul(
                            nk_ps, inv_bf[:, j:j + 1], sqt[t][:, C:2 * C],
                            start=st_, stop=sp_)
                    for pr2 in range(2):
                        jj = 4 * f + 2 * pr2 + 1
                        sS_, sP_ = (jj == 1), (jj == NT - 1)
                        for h in range(NH):
                            hs48 = slice(h * CH, (h + 1) * CH)
                            ks48 = slice(C + h * CH, C + (h + 1) * CH)
                            nc.tensor.matmul(
                                ps_s[:, h, :],
                                qsc8p[pr2][:, :, hs48],
                                qk8p[pr2][:, :, ks48],
                                start=sS_, stop=sP_, perf_mode=DR)

                # ---------------- attention + G build --------------------
                nqrow = work.tile([1, C], f32, tag="nqrow", bufs=1)
                nc.vector.tensor_copy(nqrow[:], nq_ps)
                nc.sync.dma_start(nq_dram[:], nqrow[:])
                rqk = work.tile([CH, NH], f32, tag="rqk", bufs=1)
                nc.gpsimd.dma_start(
                    rqk[:], nq_dram.rearrange("a (h d) -> d (a h)", d=CH))
                rkrow = work.tile([1, C], f32, tag="rkrow", bufs=1)
                nc.scalar.activation(rkrow[:], nk_ps, AF.Sqrt)
                nc.vector.tensor_scalar_max(rkrow[:], rkrow[:], EPS_NORM)
                rki = work.tile([1, C], f32, tag="rki", bufs=1)
                nc.vector.reciprocal(rki[:], rkrow[:])
                rk_bf = work.tile([1, C], bf16, tag="rkbf", bufs=1)
                nc.vector.tensor_copy(rk_bf[:], rki[:])
                rkb = ps.tile([CH, C], f32, tag="ps")
                nc.tensor.matmul(
                    rkb[:], onesrow_bf[0:1, 0:CH], rk_bf[:],
                    start=True, stop=True)
                rqc = work.tile([CH, NH], f32, tag="rqc", bufs=1)
                nc.scalar.activation(rqc[:], rqk[:], AF.Sqrt)
                nc.vector.tensor_scalar_max(rqc[:], rqc[:], EPS_NORM)
                rqi = work.tile([CH, NH], f32, tag="rqi", bufs=1)
                nc.vector.reciprocal(rqi[:], rqc[:])
                nc.vector.tensor_mul(rqi[:], rqi[:], schb[:])
                sS = work.tile([CH, NH, CH], f32, tag="sS", bufs=1)
                nc.vector.tensor_mul(
                    sS[:], ps_s[:],
                    rqi[:, :, None].to_broadcast((CH, NH, CH)))
                rkb3 = rkb.rearrange("d (h e) -> d h e", e=CH)
                nc.vector.tensor_mul(sS[:], sS[:], rkb3)
                expS = work.tile([CH, NH, CH], f32, tag="expS", bufs=1)
                nc.scalar.activation(expS[:], sS[:], AF.Exp)
                esum = work.tile([CH, NH, 1], f32, tag="esum", bufs=1)
                nc.vector.reduce_sum(esum[:], expS[:], axis=AX.X)
                esi = work.tile([CH, NH, 1], f32, tag="esi", bufs=1)
                nc.vector.reciprocal(esi[:], esum[:])
                attn_bf = work.tile([CH, NH, CH], bf16, tag="attnb", bufs=1)
                nc.vector.tensor_mul(
                    attn_bf[:], expS[:], esi.to_broadcast((CH, NH, CH)))
                m1 = work.tile([CH, NH, C], bf16, tag="m1", bufs=1)
                for h in range(NH):
                    pm = ps.tile([CH, C], f32, tag="ps")
                    nc.tensor.matmul(
                        pm[:], attn_bf[:, h, :], wpj_sb[:, h, :],
                        start=True, stop=True)
                    nc.vector.tensor_copy(m1[:, h, :], pm[:])
                gbf = resA.tile([P, 4, C], f8, tag="gbf", bufs=2)
                nc.gpsimd.memset(gbf[:, 3, :], 0.0)
                for jc in range(KS):
                    pg = ps.tile([P, C], f32, tag="ps")
                    for h in range(NH):
                        nc.tensor.matmul(
                            pg[:], wv_sb[:, h, jc * P:(jc + 1) * P],
                            m1[:, h, :], start=(h == 0), stop=(h == NH - 1))
                    nc.vector.tensor_scalar(
                        gbf[:, jc, :], pg[:], 64.0, None, op0=ALU.mult)
                rstdc = work.tile([P, NT], bf16, tag="rstdc", bufs=1)
                nc.scalar.activation(rstdc[:], invcol[:], AF.Sqrt)
                psT = ps.tile([NT, P], bf16, tag="ps")
                nc.tensor.transpose(psT[:], rstdc[:], ident_bf[:])
                rstdT = work.tile([NT, P], bf16, tag="rstdT", bufs=1)
                nc.vector.tensor_copy(rstdT[:], psT[:])
                rstd_row = resA.tile([1, NT, P], bf16, tag="rstdrow", bufs=2)
                nc.gpsimd.dma_start(rstd_row[:], rstdT[:])

                # ---------------- pass B1: y = x + attn branch + stats ---
                ybf = resB.tile([P, KS, N], bf16, tag="ybf")

                def emit_b1_stats(f):
                    sl = slice(f * FG, (f + 1) * FG)
                    ysq = work.tile([P, KS, FG], bf16, tag="ysq",
                                    name=f"ysq{f}")
                    nc.vector.tensor_mul(ysq[:], ybf[:, :, sl], ybf[:, :, sl])
                    pst_a = ps.tile([1, FG], f32, tag="ps", name=f"bsta{f}")
                    pst_b = ps.tile([1, FG], f32, tag="ps", name=f"bstb{f}")
                    for s in range(KS):
                        nc.tensor.matmul(
                            pst_a[:], ones_bf[:], ybf[:, s, sl],
                            start=(s == 0), stop=(s == KS - 1))
                    for s in range(KS):
                        nc.tensor.matmul(
                            pst_b[:], ones_bf[:], ysq[:, s, :],
                            start=(s == 0), stop=(s == KS - 1))
                    srow2 = work.tile([1, 2, FG], f32, tag="srow",
                                      name=f"srow2{f}")
                    nc.scalar.copy(srow2[0:1, 0, :], pst_a[:])
                    nc.scalar.copy(srow2[0:1, 1, :], pst_b[:])
                    nc.sync.dma_start(st2_dram[:, sl], srow2[:])

                for f in range(NFG):
                    sl = slice(f * FG, (f + 1) * FG)
                    xc2 = work.tile([P, KS, FG], f32, tag="xcf", bufs=3,
                                    name=f"xc2{f}")
                    nc.sync.dma_start(xc2[:], xs_r[img][:, :, sl])
                    psR = ps.tile([P, FG], f32, tag="ps")
                    nc.tensor.matmul(
                        psR[:], onesrow_bf[:],
                        rstd_row.rearrange("a j p -> a (j p)")[:, sl],
                        start=True, stop=True)
                    rb_sb = work.tile([P, FG], bf16, tag="rbsb", bufs=1)
                    nc.vector.tensor_scalar(
                        rb_sb[:], psR[:], 1.0 / 64.0, None, op0=ALU.mult)
                    x8b = work.tile([P, 4, FG], f8, tag="x8")
                    if f < 2:
                        nc.gpsimd.memset(x8b[:, 3, :], 0.0)
                    nc.vector.tensor_copy(x8b[:, 0:KS, :], xc2[:])
                    for jc in range(KS):
                        px = ps.tile([P, FG], f32, tag="ps")
                        for pr in range(2):
                            pp2 = slice(2 * pr, 2 * pr + 2)
                            nc.tensor.matmul(
                                px[:], gbf[:, pp2, jc * P:(jc + 1) * P],
                                x8b[:, pp2, :],
                                start=(pr == 0), stop=(pr == 1), perf_mode=DR)
                        nc.vector.tensor_mul(ybf[:, jc, sl], px[:], rb_sb[:])
                        nc.gpsimd.tensor_add(
                            ybf[:, jc, sl], ybf[:, jc, sl], xc2[:, jc, :])
                    if f > 0:
                        emit_b1_stats(f - 1)
                emit_b1_stats(NFG - 1)
                cst2 = work.tile([P, 2, NT], f32, tag="cst2", bufs=1)
                for kk in range(2):
                    nc.gpsimd.dma_start(
                        cst2[:, kk, :],
                        st2_dram[kk, :].rearrange("(j p) -> p j", p=P))
                mr2 = work.tile([P, 2, NT], f32, tag="mr2", bufs=1)
                nc.vector.tensor_scalar(
                    mr2[:, 0, :], cst2[:, 0, :], -1.0 / C, None, op0=ALU.mult)
                v2 = work.tile([P, NT], f32, tag="v2", bufs=1)
                nc.vector.tensor_scalar(
                    v2[:], cst2[:, 1, :], 1.0 / C, EPS_LN,
                    op0=ALU.mult, op1=ALU.add)
                msq2 = work.tile([P, NT], f32, tag="msq2", bufs=1)
                nc.vector.tensor_mul(msq2[:], mr2[:, 0, :], mr2[:, 0, :])
                nc.vector.tensor_sub(v2[:], v2[:], msq2[:])
                vi2 = work.tile([P, NT], f32, tag="vi2", bufs=1)
                nc.vector.reciprocal(vi2[:], v2[:])
                nc.scalar.activation(mr2[:, 1, :], vi2[:], AF.Sqrt, scale=256.0)
                nc.vector.tensor_mul(mr2[:, 0, :], mr2[:, 0, :], mr2[:, 1, :])
                mr2_bf = work.tile([P, 2, NT], bf16, tag="mr2b", bufs=1)
                nc.vector.tensor_copy(mr2_bf[:], mr2[:])
                psT2 = ps.tile([2 * NT, P], bf16, tag="ps")
                nc.tensor.transpose(
                    psT2[:], mr2_bf.rearrange("p two j -> p (two j)"),
                    ident_bf[:])
                m2T = work.tile([2 * NT, P], bf16, tag="m2T", bufs=1)
                nc.vector.tensor_copy(m2T[:], psT2[:])
                m2_row = resB.tile([1, 2, NT, P], bf16, tag="m2row", bufs=1)
                nc.gpsimd.dma_start(m2_row[:], m2T[:])

                # ---------------- pass B2: LN2 + FFN + residual ----------
                yn_tiles = {}

                def emit_ynprep(f):
                    sl = slice(f * FG, (f + 1) * FG)
                    bcM = ps.tile([P, FG], f32, tag="ps", name=f"bcM{f}")
                    bcR = ps.tile([P, FG], f32, tag="ps", name=f"bcR{f}")
                    m2f = m2_row.rearrange("a two j p -> a two (j p)")
                    nc.tensor.matmul(
                        bcM[:], onesrow_bf[:], m2f[:, 0, sl],
                        start=True, stop=True)
                    nc.tensor.matmul(
                        bcR[:], onesrow_bf[:], m2f[:, 1, sl],
                        start=True, stop=True)
                    t_yn = work.tile([P, KS, FG], bf16, tag="tyn", bufs=2,
                                     name=f"tyn{f}")
                    nc.vector.tensor_mul(
                        t_yn[:], ybf[:, :, sl],
                        bcR[:, None, :].to_broadcast((P, KS, FG)))
                    yn = work.tile([P, 4, FG], f8, tag="yn", name=f"yn{f}")
                    if img == 0 and f < 2:
                        nc.gpsimd.memset(yn[:, 3, :], 0.0)
                    nc.vector.tensor_add(
                        yn[:, 0:KS, :], t_yn[:],
                        bcM[:, None, :].to_broadcast((P, KS, FG)))
                    yn_tiles[f] = yn

                emit_ynprep(0)
                for f in range(NFG):
                    sl = slice(f * FG, (f + 1) * FG)
                    yn = yn_tiles.pop(f)
                    h_f8 = work.tile([P, KH, FG], f8, tag="h", bufs=1)
                    po_t = [ps.tile([P, FG], f32, tag="po", bufs=2,
                                    name=f"po{o}")
                            for o in range(2)]

                    def ffn2_pair(j2):
                        for o in range(2):
                            nc.tensor.matmul(
                                po_t[o][:],
                                w2_sb[:, 2 * j2:2 * j2 + 2, o * P:(o + 1) * P],
                                h_f8[:, 2 * j2:2 * j2 + 2, :],
                                start=(j2 == 0), stop=(j2 == KH // 2 - 1),
                                perf_mode=DR)

                    for m in range(KH):
                        ph = ps.tile([P, FG], f32, tag="ps")
                        for pr in range(2):
                            nc.tensor.matmul(
                                ph[:],
                                w1_sb[:, 2 * pr:2 * pr + 2, m * P:(m + 1) * P],
                                yn[:, 2 * pr:2 * pr + 2, :],
                                start=(pr == 0), stop=(pr == 1),
                                perf_mode=DR)
                        nc.scalar.activation(
                            h_f8[:, m, :], ph[:], AF.Gelu, scale=1.0 / 256.0)
                        if m == 6 and f + 1 < NFG:
                            emit_ynprep(f + 1)
                        if m >= 3 and (m - 3) % 2 == 0:
                            ffn2_pair((m - 3) // 2)
                    ffn2_pair(KH // 2 - 1)
                    po2 = ps.tile([P, FG], f32, tag="ps")
                    for j2 in range(KH // 2):
                        nc.tensor.matmul(
                            po2[:], w2_sb[:, 2 * j2:2 * j2 + 2, 2 * P:3 * P],
                            h_f8[:, 2 * j2:2 * j2 + 2, :],
                            start=(j2 == 0), stop=(j2 == KH // 2 - 1),
                            perf_mode=DR)
                    out_t = work.tile([P, KS, FG], f32, tag="xcf", bufs=3)
                    for o in range(2):
                        nc.vector.scalar_tensor_tensor(
                            out_t[:, o, :], po_t[o][:], 1.0 / 16.0,
                            ybf[:, o, sl], op0=ALU.mult, op1=ALU.add)
                    nc.vector.scalar_tensor_tensor(
                        out_t[:, 2, :], po2[:], 1.0 / 16.0,
                        ybf[:, 2, sl], op0=ALU.mult, op1=ALU.add)
                    nc.sync.dma_start(out_r[img][:, :, sl], out_t[:])
    return _split_waits(nc)


def _prep_weights(inputs):
    bf = ml_dtypes.bfloat16
    f8 = ml_dtypes.float8_e4m3fn
    w_qkv = np.asarray(inputs["w_qkv"], np.float64)
    g1 = np.asarray(inputs["g1"], np.float64)
    g2 = np.asarray(inputs["g2"], np.float64)
    for name in ("beta1", "beta2", "b_qkv", "b_proj", "b_ffn1", "b_ffn2"):
        assert not np.any(np.asarray(inputs[name])), f"{name} nonzero unsupported"
    wg = w_qkv * g1[None, :]
    wg = wg - wg.mean(axis=1, keepdims=True)  # fold LN mean-subtraction
    wg3 = wg.reshape(NH, 3 * CH, C)
    wq = wg3[:, 0:CH, :]
    wk = wg3[:, CH:2 * CH, :]
    wv_ = wg3[:, 2 * CH:3 * CH, :]
    # qk columns: all q heads first (384), then all k heads (384)
    wqk = np.concatenate(
        [wq.reshape(C, C), wk.reshape(C, C)], axis=0)  # [768, 384]
    wqk_r = np.zeros((P, 4, 2 * C), np.float64)  # K padded 384 -> 512
    wqk_r[:, 0:KS, :] = (16.0 * wqk).T.reshape(KS, P, 2 * C).transpose(1, 0, 2)
    wv_t = np.ascontiguousarray(wv_.transpose(1, 0, 2))  # [48, NH, 384]
    wpj = np.ascontiguousarray(
        np.asarray(inputs["w_proj"], np.float64).T.reshape(NH, CH, C)
        .transpose(1, 0, 2))  # [d, h, o]
    w1g = np.asarray(inputs["w_ffn1"], np.float64) * g2[None, :]
    w1g = w1g - w1g.mean(axis=1, keepdims=True)
    w1_r = np.zeros((P, 4, HID), np.float64)  # K padded 384 -> 512
    w1_r[:, 0:KS, :] = (16.0 * w1g).T.reshape(KS, P, HID).transpose(1, 0, 2)
    w2_r = np.ascontiguousarray(
        16.0 * np.asarray(inputs["w_ffn2"], np.float64).T
        .reshape(KH, P, C).transpose(1, 0, 2))  # [128, 12, 384]
    ls = np.asarray(inputs["logit_scale"], np.float32).reshape(NH)
    scale_row = np.exp(np.minimum(ls, LOGIT_MAX))[None, :]
    return dict(
        wqk=np.ascontiguousarray(wqk_r).astype(f8),
        wv=wv_t.astype(bf), wpj=wpj.astype(bf),
        w1=np.ascontiguousarray(w1_r).astype(f8), w2=w2_r.astype(f8),
        scale_row=np.ascontiguousarray(scale_row.astype(np.float32)))


def kernel(**inputs):
    from concourse.bass_utils import run_bass_kernel_spmd

    if "nc" not in _CACHE:
        _CACHE["nc"] = _build_nc()
    nc = _CACHE["nc"]

    x = np.asarray(inputs["x"], np.float32).reshape(B, C, N)
    wmap = _prep_weights(inputs)
    in_maps = []
    for c in range(NCORES):
        m = dict(wmap)
        m["xs"] = np.ascontiguousarray(x[c * BPC:(c + 1) * BPC])
        in_maps.append(m)
    res = run_bass_kernel_spmd(nc, in_maps, list(range(NCORES)))
    out = np.concatenate([r["out"] for r in res.results], axis=0)
    return out.reshape(B, C, 64, 64).astype(np.float32)


# revision 33
# speedup vs baseline: 1.3830x; 1.0145x over previous
"""Trainium2 Bass kernel for nn_CATransformer1 (XCiT-style channel-attention block).

v2: bf16 matmuls, LN centering folded into host-prepared weights, S-gram
weighted by inv-variance on the q side, transpose-free G build, fused
ffn1/ffn2 pipeline with F=512 moving tiles.

Sharding: data-parallel over batch. 16 images / 8 cores = 2 images per core.

Math (per image, x [C=384, N=4096]):
  LN1 gamma and the mean-subtraction are folded into the QKV weights on the
  host: W' = W*g1 - rowmean(W*g1) (exact because sum_c (x-m) = 0 per pixel).
  q,k are then produced directly from raw x; the per-pixel 1/std enters as
  a weight inv_n = 1/var_n on the pixel-contraction of the S-gram
  (S[c,d] = sum_n inv_n q_cn k_dn) and of the q/k norm sums.  Per-pixel
  stats are computed via ones-matmuls in row layout, round-tripped through
  DRAM into pixel-partition column layout for cheap vector postprocessing.
  The attention output + projection collapses into a per-image 384x384
  matrix G = Wproj @ concat_h(attn_h @ Wv_h) (Wv row-centered on the host, so
  G is automatically column-centered); pass B computes
  y = x + rstd ⊙ (G @ x) with rstd broadcast via ones-column matmuls.
  FFN: LN2 folded into W1'' = W1*g2 - rowmean likewise; yn = (y - m2)*rstd2
  materialized once per chunk in bf16; gelu on scalar engine; ffn2
  interleaved with ffn1 (lag 2) to keep the PE busy.
"""

import numpy as np
import ml_dtypes

B, C, NH, CH, N, HID = 16, 384, 8, 48, 4096, 1536
NCORES = 8
BPC = B // NCORES  # images per core
P = 128
KS = C // P    # 3 k-subtiles for C
KH = HID // P  # 12 k-subtiles for HID
FG = 512       # pixel chunk
NFG = N // FG  # 8
NT = N // P    # 32 128-pixel chunks
LOGIT_MAX = float(np.log(1.0 / 0.01))
EPS_LN = 1e-5
EPS_NORM = 1e-12

_CACHE = {}


def _patch_tile_drain():
    """Walrus in this env rejects >1 sync-wait on the kernel-tail Drain
    (CTRL_NO_STRUCT setupSyncWait).  Split the waits across a chain of
    drain instructions, one wait each.  Idempotent, in-process only."""
    import concourse.tile as tile
    from concourse import mybir
    from concourse.vector_clock import ScopedClock

    if getattr(tile.TileContext._drain_and_barrier, "_split_patch", False):
        return

    def _split_drain(self, tick_clock, wait_clock):
        drain_inst = self.nc.sync.drain()
        wait_clock.add_sem_waits(
            drain_inst.ins, ScopedClock({None: tick_clock.global_clock}))
        si = drain_inst.ins.sync_info
        if si is not None and si.on_wait and len(si.on_wait) > 1:
            waits = list(si.on_wait)
            si.on_wait = waits[:1]
            for w in waits[1:]:
                d2 = self.nc.sync.drain()
                d2.ins.sync_info = mybir.SyncInfo(on_wait=[w], on_update=[])
        self.nc.all_engine_barrier()
        popped = self.nc._tile_sem_poison_stack.pop()
        assert popped is self._sem_poison
        self.nc.clear_and_free_semaphores(list(self.sems.allocated().values()))
        self.nc.all_engine_barrier()

    _split_drain._split_patch = True
    tile.TileContext._drain_and_barrier = _split_drain


def _split_waits(nc, max_waits=1):
    """This walrus build rejects instructions carrying more than one sync
    wait ('Too many sync wait commands' / 'ISA wrong length').  Move extra
    waits onto same-engine NoOps inserted immediately before."""
    from concourse import mybir

    n = 0
    for fn in nc.m.functions:
        for blk in fn.blocks:
            out = []
            for inst in blk.instructions:
                si = inst.sync_info
                if si is not None and si.on_wait and len(si.on_wait) > max_waits:
                    waits = list(si.on_wait)
                    for w in waits[:-max_waits]:
                        n += 1
                        nop = mybir.InstNoOp(
                            name=f"I-wsplit-{n}", ins=[], outs=[])
                        nop.engine = inst.engine
                        nop.sync_info = mybir.SyncInfo(
                            on_wait=[w], on_update=[])
                        out.append(nop)
                    si.on_wait = waits[-max_waits:]
                out.append(inst)
            blk.instructions = out
    return nc


def _build_nc():
    import concourse.bass as bass
    import concourse.tile as tile
    from concourse import mybir
    from concourse.masks import make_identity

    dt = mybir.dt
    AF = mybir.ActivationFunctionType
    ALU = mybir.AluOpType
    AX = mybir.AxisListType

    f32 = dt.float32
    bf16 = dt.bfloat16
    f8 = dt.float8e4
    DR = mybir.MatmulPerfMode.DoubleRow

    _patch_tile_drain()
    nc = bass.Bass()

    xs = nc.declare_dram_parameter("xs", [BPC, C, N], f32, isOutput=False)
    wqk_d = nc.declare_dram_parameter("wqk", [P, 4, 2 * C], f8, isOutput=False)
    wv_d = nc.declare_dram_parameter("wv", [CH, NH, C], bf16, isOutput=False)
    wpj_d = nc.declare_dram_parameter("wpj", [CH, NH, C], bf16, isOutput=False)
    w1_d = nc.declare_dram_parameter("w1", [P, 4, HID], f8, isOutput=False)
    w2_d = nc.declare_dram_parameter("w2", [P, KH, C], f8, isOutput=False)
    scale_d = nc.declare_dram_parameter("scale_row", [1, NH], f32, isOutput=False)
    out_d = nc.declare_dram_parameter("out", [BPC, C, N], f32, isOutput=True)

    with tile.TileContext(nc) as tc:
        with (
            tc.tile_pool(name="consts", bufs=1) as consts,
            tc.tile_pool(name="resA", bufs=1) as resA,
            tc.tile_pool(name="resB", bufs=1) as resB,
            tc.tile_pool(name="work", bufs=2) as work,
            tc.tile_pool(name="ps", bufs=4, space="PSUM") as ps,
            tc.tile_pool(name="psacc", bufs=1, space="PSUM") as psacc,
            tc.tile_pool(name="dram", bufs=2, space="DRAM") as dramp,
        ):
            def bcast_read(dst, dram_row, parts):
                src = bass.AP(
                    tensor=dram_row.tensor, offset=dram_row.offset,
                    ap=[[0, parts]] + [list(d) for d in dram_row.ap[-1:]])
                nc.gpsimd.dma_start(dst, src)

            # ----------------- constants -----------------
            wqk_sb = consts.tile([P, 4, 2 * C], f8, tag="wqk")
            nc.scalar.dma_start(wqk_sb[:], wqk_d[:])
            wv_sb = consts.tile([CH, NH, C], bf16, tag="wv")
            nc.scalar.dma_start(wv_sb[:], wv_d[:])
            wpj_sb = consts.tile([CH, NH, C], bf16, tag="wpj")
            nc.scalar.dma_start(wpj_sb[:], wpj_d[:])
            w1_sb = consts.tile([P, 4, HID], f8, tag="w1")
            nc.scalar.dma_start(w1_sb[:], w1_d[:])
            w2_sb = consts.tile([P, KH, C], f8, tag="w2")
            nc.scalar.dma_start(w2_sb[:], w2_d[:])
            ones_f = consts.tile([P, 1], f32, tag="onesf")
            nc.vector.memset(ones_f[:], 1.0)
            ones_bf = consts.tile([P, 1], bf16, tag="ones")
            nc.vector.tensor_copy(ones_bf[:], ones_f[:])
            onesrow_f = consts.tile([1, P], f32, tag="onesrowf")
            nc.vector.memset(onesrow_f[:], 1.0)
            onesrow_bf = consts.tile([1, P], bf16, tag="onesrow")
            nc.vector.tensor_copy(onesrow_bf[:], onesrow_f[:])
            ident_bf = consts.tile([P, P], bf16, tag="ident")
            make_identity(nc, ident_bf[:])
            ones_f8 = consts.tile([P, 1], f8, tag="ones8")
            nc.vector.tensor_copy(ones_f8[:], ones_f[:])
            schb = consts.tile([CH, NH], f32, tag="schb")
            bcast_read(schb[:], scale_d[0, :], parts=CH)

            xs_r = xs.rearrange("b (s p) n -> b p s n", p=P)
            out_r = out_d.rearrange("b (s p) n -> b p s n", p=P)

            for img in range(BPC):
                st_dram = dramp.tile([2, N], f32, tag="st")
                st2_dram = dramp.tile([2, N], f32, tag="st2")
                nq_dram = dramp.tile([1, C], f32, tag="nq")

                invcol = resA.tile([P, NT], f32, tag="invc", bufs=2)
                inv_bf = resA.tile([P, NT], bf16, tag="invb", bufs=2)
                ps_s = psacc.tile([CH, NH, CH], f32, tag="S")
                norms = psacc.tile([33, C], f32, tag="N")
                nq_ps = norms[0:1, :]
                nk_ps = norms[32:33, :]

                # ---------------- pass A: stats + qk + S/norm accum ------
                xc_t, x8_t = {}, {}

                def emit_xc(f):
                    sl = slice(f * FG, (f + 1) * FG)
                    xc = work.tile([P, KS, FG], f32, tag="xcf", bufs=3,
                                   name=f"xc{f}")
                    nc.sync.dma_start(xc[:], xs_r[img][:, :, sl])
                    xc_t[f] = xc

                def emit_cast(f):
                    xc = xc_t.pop(f)
                    x8 = work.tile([P, 4, FG], f8, tag="x8", name=f"x8_{f}")
                    if img == 0 and f < 2:
                        nc.gpsimd.memset(x8[:, 3, :], 0.0)
                    nc.vector.tensor_copy(x8[:, 0:KS, :], xc[:])
                    xsq = work.tile([P, 4, FG], f8, tag="xsq", bufs=2,
                                    name=f"xsq{f}")
                    nc.vector.tensor_mul(xsq[:], x8[:], x8[:])
                    x8_t[f] = (x8, xsq)

                emit_xc(0)
                emit_xc(1)
                emit_cast(0)
                norm_pend = {}
                for f in range(NFG):
                    sl = slice(f * FG, (f + 1) * FG)
                    if f + 2 < NFG:
                        emit_xc(f + 2)
                    if f + 1 < NFG:
                        emit_cast(f + 1)
                    x8, xsq = x8_t.pop(f)
                    pst_a = ps.tile([1, FG], f32, tag="ps")
                    pst_b = ps.tile([1, FG], f32, tag="ps")
                    for s in range(KS):
                        nc.tensor.matmul(
                            pst_a[:], ones_f8[:], x8[:, s, :],
                            start=(s == 0), stop=(s == KS - 1))
                    for s in range(KS):
                        nc.tensor.matmul(
                            pst_b[:], ones_f8[:], xsq[:, s, :],
                            start=(s == 0), stop=(s == KS - 1))
                    srow = work.tile([1, 2, FG], f32, tag="srow")
                    nc.scalar.copy(srow[0:1, 0, :], pst_a[:])
                    nc.scalar.copy(srow[0:1, 1, :], pst_b[:])
                    nc.sync.dma_start(st_dram[:, sl], srow[:])
                    cstat = work.tile([P, 2, 4], f32, tag="cst")
                    for kk in range(2):
                        nc.gpsimd.dma_start(
                            cstat[:, kk, :],
                            st_dram[kk, sl].rearrange("(j p) -> p j", p=P))
                    mcol = work.tile([P, 4], f32, tag="mcol")
                    nc.vector.tensor_scalar(
                        mcol[:], cstat[:, 0, :], 1.0 / C, None, op0=ALU.mult)
                    vcol = work.tile([P, 4], f32, tag="vcol")
                    nc.vector.tensor_scalar(
                        vcol[:], cstat[:, 1, :], 1.0 / C, EPS_LN,
                        op0=ALU.mult, op1=ALU.add)
                    nc.vector.tensor_mul(mcol[:], mcol[:], mcol[:])
                    nc.vector.tensor_sub(vcol[:], vcol[:], mcol[:])
                    c4 = slice(4 * f, 4 * f + 4)
                    nc.vector.reciprocal(invcol[:, c4], vcol[:])
                    nc.vector.tensor_copy(inv_bf[:, c4], invcol[:, c4])

                    qk8p = [None, None]
                    qsc8p = [None, None]
                    sqt = [None] * 4
                    for t in range(4):
                        j = 4 * f + t
                        pr2 = t // 2
                        par = t % 2
                        if par == 0:
                            qk8p[pr2] = work.tile(
                                [P, 2, 2 * C], f8, tag="qk8p", bufs=2,
                                name=f"qk8p{f}_{pr2}")
                            qsc8p[pr2] = work.tile(
                                [P, 2, C], f8, tag="qsc8", bufs=2,
                                name=f"qsc8p{f}_{pr2}")
                        qk8raw, qsc8 = qk8p[pr2], qsc8p[pr2]
                        pa = ps.tile([P, 512], f32, tag="ps")
                        pb = ps.tile([P, 256], f32, tag="ps")
                        lsl = slice(t * P, (t + 1) * P)
                        for pr in range(2):
                            pp2 = slice(2 * pr, 2 * pr + 2)
                            nc.tensor.matmul(
                                pa[:], x8[:, pp2, lsl], wqk_sb[:, pp2, 0:512],
                                start=(pr == 0), stop=(pr == 1), perf_mode=DR)
                            nc.tensor.matmul(
                                pb[:], x8[:, pp2, lsl], wqk_sb[:, pp2, 512:768],
                                start=(pr == 0), stop=(pr == 1), perf_mode=DR)
                        nc.vector.tensor_scalar(
                            qk8raw[:, par, 0:512], pa[:], 1.0 / 16.0,
                            None, op0=ALU.mult)
                        nc.vector.tensor_scalar(
                            qk8raw[:, par, 512:768], pb[:], 1.0 / 16.0,
                            None, op0=ALU.mult)
                        nc.vector.tensor_scalar(
                            qsc8[:, par, :], pa[:, 0:C], invcol[:, j:j + 1],
                            1.0 / 16.0, op0=ALU.mult, op1=ALU.mult)
                        sq_bf = work.tile([P, 2 * C], bf16, tag="sqbf",
                                          bufs=8, name=f"sq{f}_{t}")
                        nc.gpsimd.tensor_mul(
                            sq_bf[:], qk8raw[:, par, :], qk8raw[:, par, :])
                        sqt[t] = sq_bf
                    norm_pend[f] = sqt

                    def emit_norms(fn):
                        for t in range(4):
                            j = 4 * fn + t
                            st_, sp_ = (j == 0), (j == NT - 1)
                            nc.tensor.matmul(
                                nq_ps, inv_bf[:, j:j + 1],
                                norm_pend[fn][t][:, 0:C],
                                start=st_, stop=sp_)
                            nc.tensor.matmul(
                                nk_ps, inv_bf[:, j:j + 1],
                                norm_pend[fn][t][:, C:2 * C],
                                start=st_, stop=sp_)
                        del norm_pend[fn]

                    if f > 0:
                        emit_norms(f - 1)
                    for pr2 in range(2):
                        jj = 4 * f + 2 * pr2 + 1
                        sS_, sP_ = (jj == 1), (jj == NT - 1)
                        for h in range(NH):
                            hs48 = slice(h * CH, (h + 1) * CH)
                            ks48 = slice(C + h * CH, C + (h + 1) * CH)
                            nc.tensor.matmul(
                                ps_s[:, h, :],
                                qsc8p[pr2][:, :, hs48],
                                qk8p[pr2][:, :, ks48],
                                start=sS_, stop=sP_, perf_mode=DR)

                emit_norms(NFG - 1)

                # ---------------- attention + G build --------------------
                # rstd chain first: only needs invcol, overlaps the norms
                # roundtrip + softmax serial region
                rstdc = work.tile([P, NT], bf16, tag="rstdc", bufs=1)
                nc.scalar.activation(rstdc[:], invcol[:], AF.Sqrt)
                psT = ps.tile([NT, P], bf16, tag="ps")
                nc.tensor.transpose(psT[:], rstdc[:], ident_bf[:])
                rstdT = work.tile([NT, P], bf16, tag="rstdT", bufs=1)
                nc.vector.tensor_copy(rstdT[:], psT[:])
                rstd_row = resA.tile([1, NT, P], bf16, tag="rstdrow", bufs=2)
                nc.gpsimd.dma_start(rstd_row[:], rstdT[:])
                nqrow = work.tile([1, C], f32, tag="nqrow", bufs=1)
                nc.vector.tensor_copy(nqrow[:], nq_ps)
                nc.sync.dma_start(nq_dram[:], nqrow[:])
                rqk = work.tile([CH, NH], f32, tag="rqk", bufs=1)
                nc.gpsimd.dma_start(
                    rqk[:], nq_dram.rearrange("a (h d) -> d (a h)", d=CH))
                rkrow = work.tile([1, C], f32, tag="rkrow", bufs=1)
                nc.scalar.activation(rkrow[:], nk_ps, AF.Sqrt)
                nc.vector.tensor_scalar_max(rkrow[:], rkrow[:], EPS_NORM)
                rki = work.tile([1, C], f32, tag="rki", bufs=1)
                nc.vector.reciprocal(rki[:], rkrow[:])
                rk_bf = work.tile([1, C], bf16, tag="rkbf", bufs=1)
                nc.vector.tensor_copy(rk_bf[:], rki[:])
                rkb = ps.tile([CH, C], f32, tag="ps")
                nc.tensor.matmul(
                    rkb[:], onesrow_bf[0:1, 0:CH], rk_bf[:],
                    start=True, stop=True)
                rqc = work.tile([CH, NH], f32, tag="rqc", bufs=1)
                nc.scalar.activation(rqc[:], rqk[:], AF.Sqrt)
                nc.vector.tensor_scalar_max(rqc[:], rqc[:], EPS_NORM)
                rqi = work.tile([CH, NH], f32, tag="rqi", bufs=1)
                nc.vector.reciprocal(rqi[:], rqc[:])
                nc.vector.tensor_mul(rqi[:], rqi[:], schb[:])
                sS = work.tile([CH, NH, CH], f32, tag="sS", bufs=1)
                nc.vector.tensor_mul(
                    sS[:], ps_s[:],
                    rqi[:, :, None].to_broadcast((CH, NH, CH)))
                rkb3 = rkb.rearrange("d (h e) -> d h e", e=CH)
                nc.vector.tensor_mul(sS[:], sS[:], rkb3)
                expS = work.tile([CH, NH, CH], f32, tag="expS", bufs=1)
                nc.scalar.activation(expS[:], sS[:], AF.Exp)
                esum = work.tile([CH, NH, 1], f32, tag="esum", bufs=1)
                nc.vector.reduce_sum(esum[:], expS[:], axis=AX.X)
                esi = work.tile([CH, NH, 1], f32, tag="esi", bufs=1)
                nc.vector.reciprocal(esi[:], esum[:])
                attn_bf = work.tile([CH, NH, CH], bf16, tag="attnb", bufs=1)
                nc.vector.tensor_mul(
                    attn_bf[:], expS[:], esi.to_broadcast((CH, NH, CH)))
                m1 = work.tile([CH, NH, C], bf16, tag="m1", bufs=1)
                for h in range(NH):
                    pm = ps.tile([CH, C], f32, tag="ps")
                    nc.tensor.matmul(
                        pm[:], attn_bf[:, h, :], wpj_sb[:, h, :],
                        start=True, stop=True)
                    nc.scalar.copy(m1[:, h, :], pm[:])
                gbf = resA.tile([P, 4, C], f8, tag="gbf", bufs=2)
                nc.gpsimd.memset(gbf[:, 3, :], 0.0)
                for jc in range(KS):
                    pg = ps.tile([P, C], f32, tag="ps")
                    for h in range(NH):
                        nc.tensor.matmul(
                            pg[:], wv_sb[:, h, jc * P:(jc + 1) * P],
                            m1[:, h, :], start=(h == 0), stop=(h == NH - 1))
                    nc.scalar.mul(gbf[:, jc, :], pg[:], 64.0)
                # ---------------- pass B1: y = x + attn branch + stats ---
                ybf = resB.tile([P, KS, N], bf16, tag="ybf")

                def emit_b1_stats(f):
                    sl = slice(f * FG, (f + 1) * FG)
                    ysq = work.tile([P, KS, FG], bf16, tag="ysq",
                                    name=f"ysq{f}")
                    nc.vector.tensor_mul(ysq[:], ybf[:, :, sl], ybf[:, :, sl])
                    pst_a = ps.tile([1, FG], f32, tag="ps", name=f"bsta{f}")
                    pst_b = ps.tile([1, FG], f32, tag="ps", name=f"bstb{f}")
                    for s in range(KS):
                        nc.tensor.matmul(
                            pst_a[:], ones_bf[:], ybf[:, s, sl],
                            start=(s == 0), stop=(s == KS - 1))
                    for s in range(KS):
                        nc.tensor.matmul(
                            pst_b[:], ones_bf[:], ysq[:, s, :],
                            start=(s == 0), stop=(s == KS - 1))
                    srow2 = work.tile([1, 2, FG], f32, tag="srow",
                                      name=f"srow2{f}")
                    nc.scalar.copy(srow2[0:1, 0, :], pst_a[:])
                    nc.scalar.copy(srow2[0:1, 1, :], pst_b[:])
                    nc.sync.dma_start(st2_dram[:, sl], srow2[:])

                for f in range(NFG):
                    sl = slice(f * FG, (f + 1) * FG)
                    xc2 = work.tile([P, KS, FG], f32, tag="xcf", bufs=3,
                                    name=f"xc2{f}")
                    nc.sync.dma_start(xc2[:], xs_r[img][:, :, sl])
                    psR = ps.tile([P, FG], f32, tag="ps")
                    nc.tensor.matmul(
                        psR[:], onesrow_bf[:],
                        rstd_row.rearrange("a j p -> a (j p)")[:, sl],
                        start=True, stop=True)
                    rb_sb = work.tile([P, FG], bf16, tag="rbsb", bufs=1)
                    nc.vector.tensor_scalar(
                        rb_sb[:], psR[:], 1.0 / 64.0, None, op0=ALU.mult)
                    x8b = work.tile([P, 4, FG], f8, tag="x8")
                    if f < 2:
                        nc.gpsimd.memset(x8b[:, 3, :], 0.0)
                    nc.vector.tensor_copy(x8b[:, 0:KS, :], xc2[:])
                    for jc in range(KS):
                        px = ps.tile([P, FG], f32, tag="ps")
                        for pr in range(2):
                            pp2 = slice(2 * pr, 2 * pr + 2)
                            nc.tensor.matmul(
                                px[:], gbf[:, pp2, jc * P:(jc + 1) * P],
                                x8b[:, pp2, :],
                                start=(pr == 0), stop=(pr == 1), perf_mode=DR)
                        nc.vector.tensor_mul(ybf[:, jc, sl], px[:], rb_sb[:])
                        nc.gpsimd.tensor_add(
                            ybf[:, jc, sl], ybf[:, jc, sl], xc2[:, jc, :])
                    if f > 0:
                        emit_b1_stats(f - 1)
                emit_b1_stats(NFG - 1)
                cst2 = work.tile([P, 2, NT], f32, tag="cst2", bufs=1)
                for kk in range(2):
                    nc.gpsimd.dma_start(
                        cst2[:, kk, :],
                        st2_dram[kk, :].rearrange("(j p) -> p j", p=P))
                mr2 = work.tile([P, 2, NT], f32, tag="mr2", bufs=1)
                nc.vector.tensor_scalar(
                    mr2[:, 0, :], cst2[:, 0, :], -1.0 / C, None, op0=ALU.mult)
                v2 = work.tile([P, NT], f32, tag="v2", bufs=1)
                nc.vector.tensor_scalar(
                    v2[:], cst2[:, 1, :], 1.0 / C, EPS_LN,
                    op0=ALU.mult, op1=ALU.add)
                msq2 = work.tile([P, NT], f32, tag="msq2", bufs=1)
                nc.vector.tensor_mul(msq2[:], mr2[:, 0, :], mr2[:, 0, :])
                nc.vector.tensor_sub(v2[:], v2[:], msq2[:])
                vi2 = work.tile([P, NT], f32, tag="vi2", bufs=1)
                nc.vector.reciprocal(vi2[:], v2[:])
                nc.scalar.activation(mr2[:, 1, :], vi2[:], AF.Sqrt, scale=256.0)
                nc.vector.tensor_mul(mr2[:, 0, :], mr2[:, 0, :], mr2[:, 1, :])
                mr2_bf = work.tile([P, 2, NT], bf16, tag="mr2b", bufs=1)
                nc.vector.tensor_copy(mr2_bf[:], mr2[:])
                psT2 = ps.tile([2 * NT, P], bf16, tag="ps")
                nc.tensor.transpose(
                    psT2[:], mr2_bf.rearrange("p two j -> p (two j)"),
                    ident_bf[:])
                m2T = work.tile([2 * NT, P], bf16, tag="m2T", bufs=1)
                nc.vector.tensor_copy(m2T[:], psT2[:])
                m2_row = resB.tile([1, 2, NT, P], bf16, tag="m2row", bufs=1)
                nc.gpsimd.dma_start(m2_row[:], m2T[:])

                # ---------------- pass B2: LN2 + FFN + residual ----------
                yn_tiles = {}

                def emit_ynprep(f):
                    sl = slice(f * FG, (f + 1) * FG)
                    bcM = ps.tile([P, FG], f32, tag="ps", name=f"bcM{f}")
                    bcR = ps.tile([P, FG], f32, tag="ps", name=f"bcR{f}")
                    m2f = m2_row.rearrange("a two j p -> a two (j p)")
                    nc.tensor.matmul(
                        bcM[:], onesrow_bf[:], m2f[:, 0, sl],
                        start=True, stop=True)
                    nc.tensor.matmul(
                        bcR[:], onesrow_bf[:], m2f[:, 1, sl],
                        start=True, stop=True)
                    t_yn = work.tile([P, KS, FG], bf16, tag="tyn", bufs=2,
                                     name=f"tyn{f}")
                    nc.vector.tensor_mul(
                        t_yn[:], ybf[:, :, sl],
                        bcR[:, None, :].to_broadcast((P, KS, FG)))
                    yn = work.tile([P, 4, FG], f8, tag="yn", name=f"yn{f}")
                    if img == 0 and f < 2:
                        nc.gpsimd.memset(yn[:, 3, :], 0.0)
                    nc.vector.tensor_add(
                        yn[:, 0:KS, :], t_yn[:],
                        bcM[:, None, :].to_broadcast((P, KS, FG)))
                    yn_tiles[f] = yn

                emit_ynprep(0)
                for f in range(NFG):
                    sl = slice(f * FG, (f + 1) * FG)
                    yn = yn_tiles.pop(f)
                    h_f8 = work.tile([P, KH, FG], f8, tag="h", bufs=1)
                    po_t = [ps.tile([P, FG], f32, tag="po", bufs=2,
                                    name=f"po{o}")
                            for o in range(2)]

                    def ffn2_pair(j2):
                        for o in range(2):
                            nc.tensor.matmul(
                                po_t[o][:],
                                w2_sb[:, 2 * j2:2 * j2 + 2, o * P:(o + 1) * P],
                                h_f8[:, 2 * j2:2 * j2 + 2, :],
                                start=(j2 == 0), stop=(j2 == KH // 2 - 1),
                                perf_mode=DR)

                    for m in range(KH):
                        ph = ps.tile([P, FG], f32, tag="ps")
                        for pr in range(2):
                            nc.tensor.matmul(
                                ph[:],
                                w1_sb[:, 2 * pr:2 * pr + 2, m * P:(m + 1) * P],
                                yn[:, 2 * pr:2 * pr + 2, :],
                                start=(pr == 0), stop=(pr == 1),
                                perf_mode=DR)
                        nc.scalar.activation(
                            h_f8[:, m, :], ph[:], AF.Gelu, scale=1.0 / 256.0)
                        if m == 6 and f + 1 < NFG:
                            emit_ynprep(f + 1)
                        if m >= 3 and (m - 3) % 2 == 0:
                            ffn2_pair((m - 3) // 2)
                    ffn2_pair(KH // 2 - 1)
                    po2 = ps.tile([P, FG], f32, tag="ps")
                    for j2 in range(KH // 2):
                        nc.tensor.matmul(
                            po2[:], w2_sb[:, 2 * j2:2 * j2 + 2, 2 * P:3 * P],
                            h_f8[:, 2 * j2:2 * j2 + 2, :],
                            start=(j2 == 0), stop=(j2 == KH // 2 - 1),
                            perf_mode=DR)
                    out_t = work.tile([P, KS, FG], f32, tag="xcf", bufs=3)
                    for o in range(2):
                        nc.vector.scalar_tensor_tensor(
                            out_t[:, o, :], po_t[o][:], 1.0 / 16.0,
                            ybf[:, o, sl], op0=ALU.mult, op1=ALU.add)
                    nc.vector.scalar_tensor_tensor(
                        out_t[:, 2, :], po2[:], 1.0 / 16.0,
                        ybf[:, 2, sl], op0=ALU.mult, op1=ALU.add)
                    nc.sync.dma_start(out_r[img][:, :, sl], out_t[:])
    return _split_waits(nc)


def _prep_weights(inputs):
    bf = ml_dtypes.bfloat16
    f8 = ml_dtypes.float8_e4m3fn
    w_qkv = np.asarray(inputs["w_qkv"], np.float64)
    g1 = np.asarray(inputs["g1"], np.float64)
    g2 = np.asarray(inputs["g2"], np.float64)
    for name in ("beta1", "beta2", "b_qkv", "b_proj", "b_ffn1", "b_ffn2"):
        assert not np.any(np.asarray(inputs[name])), f"{name} nonzero unsupported"
    wg = w_qkv * g1[None, :]
    wg = wg - wg.mean(axis=1, keepdims=True)  # fold LN mean-subtraction
    wg3 = wg.reshape(NH, 3 * CH, C)
    wq = wg3[:, 0:CH, :]
    wk = wg3[:, CH:2 * CH, :]
    wv_ = wg3[:, 2 * CH:3 * CH, :]
    # qk columns: all q heads first (384), then all k heads (384)
    wqk = np.concatenate(
        [wq.reshape(C, C), wk.reshape(C, C)], axis=0)  # [768, 384]
    wqk_r = np.zeros((P, 4, 2 * C), np.float64)  # K padded 384 -> 512
    wqk_r[:, 0:KS, :] = (16.0 * wqk).T.reshape(KS, P, 2 * C).transpose(1, 0, 2)
    wv_t = np.ascontiguousarray(wv_.transpose(1, 0, 2))  # [48, NH, 384]
    wpj = np.ascontiguousarray(
        np.asarray(inputs["w_proj"], np.float64).T.reshape(NH, CH, C)
        .transpose(1, 0, 2))  # [d, h, o]
    w1g = np.asarray(inputs["w_ffn1"], np.float64) * g2[None, :]
    w1g = w1g - w1g.mean(axis=1, keepdims=True)
    w1_r = np.zeros((P, 4, HID), np.float64)  # K padded 384 -> 512
    w1_r[:, 0:KS, :] = (16.0 * w1g).T.reshape(KS, P, HID).transpose(1, 0, 2)
    w2_r = np.ascontiguousarray(
        16.0 * np.asarray(inputs["w_ffn2"], np.float64).T
        .reshape(KH, P, C).transpose(1, 0, 2))  # [128, 12, 384]
    ls = np.asarray(inputs["logit_scale"], np.float32).reshape(NH)
    scale_row = np.exp(np.minimum(ls, LOGIT_MAX))[None, :]
    return dict(
        wqk=np.ascontiguousarray(wqk_r).astype(f8),
        wv=wv_t.astype(bf), wpj=wpj.astype(bf),
        w1=np.ascontiguousarray(w1_r).astype(f8), w2=w2_r.astype(f8),
        scale_row=np.ascontiguousarray(scale_row.astype(np.float32)))


def kernel(**inputs):
    from concourse.bass_utils import run_bass_kernel_spmd

    if "nc" not in _CACHE:
        _CACHE["nc"] = _build_nc()
    nc = _CACHE["nc"]

    x = np.asarray(inputs["x"], np.float32).reshape(B, C, N)
    wmap = _prep_weights(inputs)
    in_maps = []
    for c in range(NCORES):
        m = dict(wmap)
        m["xs"] = np.ascontiguousarray(x[c * BPC:(c + 1) * BPC])
        in_maps.append(m)
    res = run_bass_kernel_spmd(nc, in_maps, list(range(NCORES)))
    out = np.concatenate([r["out"] for r in res.results], axis=0)
    return out.reshape(B, C, 64, 64).astype(np.float32)
